# revision 1
# baseline (speedup 1.0000x reference)
"""GAT spatio-temporal model Trainium2 kernel (v3).

Sharding: data-parallel over batch B=8 -> 8 NeuronCores (1 graph each).
Layout: feature-on-partition ("T" tensors are [F, N]); attention computed
in transposed [m, n] layout so softmax denominators come from ones-matmul
column sums and AV products hit PE directly.

v3: per-layer two-stage structure -- stage A computes projections,
attention rows and z replications for ALL heads (deep pools so the
engines run dense, back-to-back work), stage B runs the latency-heavy
softmax/AV bodies overlapped across heads. bf16 operands on the N^2
paths, leaky-relu fused on ScalarE via Prelu+per-partition bias.

Shapes (hardcoded): B=8, N=512, Din=64, H=8, F=128, L=2.
"""
import os
import numpy as np
from contextlib import ExitStack

import concourse.bass as bass
import concourse.tile as tile
from concourse import bacc, mybir
from concourse.bass_utils import run_bass_kernel_spmd
from concourse.masks import make_identity

F32 = mybir.dt.float32
BF16 = mybir.dt.bfloat16
AF = mybir.ActivationFunctionType
OP = mybir.AluOpType

B, N, DIN, H, F, L = 8, 512, 64, 8, 128, 2
NCHUNK = N // 128  # 4
ALPHA = 0.2
LN_EPS = 1e-5
ACT_CHUNKS = int(os.environ.get("K_ACT_CHUNKS", "4"))
USE_BCAST = os.environ.get("K_BCAST", "1") == "1"

_CACHE = {}


def _bcast_row(ap_row):
    return bass.AP(tensor=ap_row.tensor, offset=ap_row.offset, ap=[[0, 128], [1, N]])


def build_nc():
    nc = bacc.Bacc("TRN2", target_bir_lowering=False, debug=False)

    x_d = nc.dram_tensor("x", [N, DIN], F32, kind="ExternalInput").ap()
    adj_d = nc.dram_tensor("adj", [N, N], mybir.dt.int32, kind="ExternalInput").ap()
    Wp_d = nc.dram_tensor("Wp", [DIN, F], F32, kind="ExternalInput").ap()
    bp_d = nc.dram_tensor("bp", [F], F32, kind="ExternalInput").ap()
    Wh_d = nc.dram_tensor("W_heads", [L, H, F, F], F32, kind="ExternalInput").ap()
    ah_d = nc.dram_tensor("a_heads", [L, H, 2 * F], F32, kind="ExternalInput").ap()
    Wo_d = nc.dram_tensor("W_out", [L, H * F, F], F32, kind="ExternalInput").ap()
    ao_d = nc.dram_tensor("a_out", [L, 2 * F], F32, kind="ExternalInput").ap()
    g_d = nc.dram_tensor("ln_g", [L, F], F32, kind="ExternalInput").ap()
    b_d = nc.dram_tensor("ln_b", [L, F], F32, kind="ExternalInput").ap()
    out_d = nc.dram_tensor("out", [N, F], F32, kind="ExternalOutput").ap()
    scr_d = [nc.dram_tensor(f"scratch{i}", [1, N], F32, kind="ExternalOutput").ap()
             for i in range(22)]

    with tile.TileContext(nc) as tc, ExitStack() as ctx:
        const = ctx.enter_context(tc.tile_pool(name="const", bufs=1))
        sproj = ctx.enter_context(tc.tile_pool(name="sproj", bufs=9))
        sprt = ctx.enter_context(tc.tile_pool(name="sprt", bufs=3))
        sbig = ctx.enter_context(tc.tile_pool(name="sbig", bufs=3))
        srow = ctx.enter_context(tc.tile_pool(name="srow", bufs=4))
        sexp_e = ctx.enter_context(tc.tile_pool(name="sexp_e", bufs=2))
        sexp_p = ctx.enter_context(tc.tile_pool(name="sexp_p", bufs=4))
        smulti = ctx.enter_context(tc.tile_pool(name="smulti", bufs=9))
        shd = ctx.enter_context(tc.tile_pool(name="shd", bufs=2))
        smask = ctx.enter_context(tc.tile_pool(name="smask", bufs=4))
        pou = ctx.enter_context(tc.tile_pool(name="pou", bufs=3, space="PSUM"))
        pmisc = ctx.enter_context(tc.tile_pool(name="pmisc", bufs=3, space="PSUM"))
        prow = ctx.enter_context(tc.tile_pool(name="prow", bufs=2, space="PSUM"))

        # ---------------- constants ----------------
        ones_row = const.tile([1, N], F32)
        nc.vector.memset(ones_row, 1.0)
        ones_col_bf = const.tile([128, 1], BF16)
        nc.vector.memset(ones_col_bf, 1.0)
        ones_col = const.tile([128, 1], F32)
        nc.vector.memset(ones_col, 1.0)
        ident = const.tile([128, 128], F32)
        make_identity(nc, ident)
        eps1 = const.tile([1, 1], F32)
        nc.vector.memset(eps1, LN_EPS)

        Wp_sb = const.tile([DIN, F], F32)
        nc.sync.dma_start(Wp_sb, Wp_d)
        bp_col = const.tile([F, 1], F32)
        nc.sync.dma_start(bp_col, bp_d.rearrange("(f one) -> f one", one=1))
        x_chunks = []
        for c in range(NCHUNK):
            xc = shd.tile([128, DIN], F32, tag="xchunk")
            nc.sync.dma_start(xc, x_d[bass.ts(c, 128), :])
            x_chunks.append(xc)

        # Per-layer weight loads on separate DMA queues: layer 0 lands
        # first (sync HWDGE) so stage A starts immediately; layer 1 and
        # W_out trickle in behind on gpsimd/scalar queues.
        Wh_all = [const.tile([F, H, F], F32, name=f"WhA{l}") for l in range(L)]
        Wh_ball = [const.tile([F, H, F], BF16, name=f"WhB{l}") for l in range(L)]
        nc.sync.dma_start(Wh_all[0], Wh_d[0].rearrange("h i o -> i h o"))
        nc.gpsimd.dma_start(Wh_all[1], Wh_d[1].rearrange("h i o -> i h o"))
        for l in range(L):
            nc.vector.tensor_copy(Wh_ball[l], Wh_all[l])
        Wh_sb = [[Wh_all[l][:, h, :] for h in range(H)] for l in range(L)]
        Wh_bf = [[Wh_ball[l][:, h, :] for h in range(H)] for l in range(L)]
        ah_all = const.tile([F, L * H, 2], F32)
        nc.sync.dma_start(ah_all, ah_d.rearrange("l h (t f) -> f (l h) t", t=2))
        ah_sb = [[ah_all[:, l * H + h, :] for h in range(H)] for l in range(L)]
        wo_f = [const.tile([128, H, F], F32, name=f"WoF{l}") for l in range(L)]
        Wo_ball = [const.tile([128, H, F], BF16, name=f"WoB{l}") for l in range(L)]
        for l in range(L):
            nc.gpsimd.dma_start(wo_f[l], Wo_d[l].rearrange("(c p) f -> p c f", p=128))
            nc.vector.tensor_copy(Wo_ball[l], wo_f[l])
        Wo_bf = [Wo_ball[l] for l in range(L)]
        ao_all = const.tile([F, L, 2], F32)
        nc.sync.dma_start(ao_all, ao_d.rearrange("l (t f) -> f l t", t=2))
        ao_sb = [ao_all[:, l, :] for l in range(L)]
        g_row = [const.tile([1, F], F32, name=f"grow_{l}") for l in range(L)]
        b_row = [const.tile([1, F], F32, name=f"brow_{l}") for l in range(L)]
        g_col = [const.tile([F, 1], F32, name=f"gcol_{l}") for l in range(L)]
        for l in range(L):
            nc.sync.dma_start(g_row[l], g_d[l].rearrange("(one f) -> one f", one=1))
            nc.sync.dma_start(b_row[l], b_d[l].rearrange("(one f) -> one f", one=1))
            nc.sync.dma_start(g_col[l], g_d[l].rearrange("(f one) -> f one", one=1))

        # ---------------- x -> xT, input projection (chunk-wise) ----------------
        xT = const.tile([DIN, N], F32)
        ph = pmisc.tile([128, N], F32, tag="pbig")
        hT = sbig.tile([128, N], F32, tag="hT")
        hT_bf = sbig.tile([128, N], BF16, tag="hTb")
        for c in range(NCHUNK):
            pt = pmisc.tile([DIN, 128], F32, tag="pbig")
            nc.tensor.transpose(pt, x_chunks[c], ident)
            nc.scalar.activation(xT[:, bass.ts(c, 128)], pt, AF.Copy)
            nc.tensor.matmul(ph[:, bass.ts(c, 128)], Wp_sb, xT[:, bass.ts(c, 128)],
                             start=True, stop=True)
            nc.scalar.activation(hT[:, bass.ts(c, 128)], ph[:, bass.ts(c, 128)],
                                 AF.Relu, bias=bp_col)
            nc.vector.tensor_copy(hT_bf[:, bass.ts(c, 128)], hT[:, bass.ts(c, 128)])

        # ---------------- adj -> maskT (bf16, transposed) ----------------
        adj_f = []
        for r in range(NCHUNK):
            ai = shd.tile([128, N], mybir.dt.int32, tag="adji")
            eng = nc.scalar if r % 2 == 0 else nc.sync
            eng.dma_start(ai, adj_d[bass.ts(r, 128), :])
            af = smask.tile([128, N], F32, tag="adjf")
            nc.vector.tensor_copy(af, ai)
            adj_f.append(af)
        maskT = [const.tile([128, N], BF16, name=f"maskT{c}") for c in range(NCHUNK)]
        # One small psum per (c,r) block, evacuated immediately: slots
        # recycle as each adj chunk lands instead of holding a bank while
        # the full 1MB adjacency streams in.
        for r in range(NCHUNK):
            for c in range(NCHUNK):
                pm = pmisc.tile([128, 128], F32, tag="pbig")
                nc.tensor.transpose(pm, adj_f[r][:, bass.ts(c, 128)], ident)
                nc.scalar.activation(maskT[c][:, bass.ts(r, 128)], pm, AF.Copy)

        # ---------------- stage helpers ----------------
        def stage_a(projT, a_cols, hid, use_bcast=True):
            """Rows + z replication for one attention. projT fp32 [F,N].
            Returns (z_sb, s2cols)."""
            s12p = prow.tile([2, N], F32, tag="prow")
            nc.tensor.matmul(s12p, a_cols, projT, start=True, stop=True)
            s12row = srow.tile([2, N], F32, tag="rowA")
            nc.vector.tensor_copy(s12row, s12p)
            z_sb = sproj.tile([128, N], F32, tag="z_sb")
            s2cols = sproj.tile([128, 4], F32, tag="s2cols")
            if use_bcast:
                nc.sync.dma_start(scr_d[hid], s12row[0:1, :])
                nc.sync.dma_start(z_sb, _bcast_row(scr_d[hid][0, :]))
            else:
                prz = pmisc.tile([128, N], F32, tag="pbig")
                nc.tensor.matmul(prz, ones_row[:, 0:128], s12row[0:1, :],
                                 start=True, stop=True)
                nc.scalar.activation(z_sb, prz, AF.Copy)
            nc.sync.dma_start(scr_d[10 + hid], s12row[1:2, :])
            s2scr = scr_d[10 + hid][0, :]
            nc.sync.dma_start(s2cols, bass.AP(tensor=s2scr.tensor, offset=s2scr.offset,
                                              ap=[[1, 128], [128, 4]]))
            return z_sb, s2cols

        def stage_b(z_sb, s2cols, projN_bf, hid, act_chunks=None, use_bcast=True):
            """Softmax + AV for one attention. Returns (pou_t, rep_sb)."""
            if act_chunks is None:
                act_chunks = ACT_CHUNKS
            e_all = sexp_e.tile([128, NCHUNK, N], F32, tag="e_all")
            p_all = sexp_p.tile([128, NCHUNK, N], BF16, tag="p_all")
            for c in range(NCHUNK):
                if c < act_chunks:
                    nc.scalar.activation(e_all[:, c, :], z_sb, AF.Prelu,
                                         bias=s2cols[:, c:c + 1], alpha=ALPHA)
                else:
                    u = shd.tile([128, N], F32, tag="lrelu_u")
                    nc.vector.tensor_scalar_add(u, z_sb, s2cols[:, c:c + 1])
                    t = shd.tile([128, N], F32, tag="lrelu_t")
                    nc.vector.tensor_scalar_mul(t, u, ALPHA)
                    nc.vector.tensor_tensor(e_all[:, c, :], u, t, OP.max)
            nc.scalar.activation(p_all, e_all, AF.Exp)
            for c in range(NCHUNK):
                nc.vector.tensor_tensor(p_all[:, c, :], p_all[:, c, :], maskT[c],
                                        OP.mult)
            pden = prow.tile([1, N], F32, tag="prow")
            pou_t = pou.tile([128, N], F32, tag="oU")
            for c in range(NCHUNK):
                nc.tensor.matmul(pden, ones_col_bf, p_all[:, c, :],
                                 start=(c == 0), stop=(c == NCHUNK - 1))
            for c in range(NCHUNK):
                nc.tensor.matmul(pou_t, projN_bf[:, bass.ts(c, 128)], p_all[:, c, :],
                                 start=(c == 0), stop=(c == NCHUNK - 1))
            r_sb = srow.tile([1, N], F32, tag="rowR")
            nc.vector.reciprocal_approx_fast(r_sb, pden)
            rep_sb = sbig.tile([128, N], F32, tag="rep")
            if use_bcast:
                nc.sync.dma_start(scr_d[8 + hid % 2], r_sb)
                nc.sync.dma_start(rep_sb, _bcast_row(scr_d[8 + hid % 2][0, :]))
            else:
                prr = pmisc.tile([128, N], F32, tag="pbig")
                nc.tensor.matmul(prr, ones_row[:, 0:128], r_sb, start=True, stop=True)
                nc.scalar.activation(rep_sb, prr, AF.Copy)
            return pou_t, rep_sb

        # ---------------- layers ----------------
        for l in range(L):
            residT = hT
            # ---- stage A: projections + rows for all heads ----
            hprojN = []
            zs = []
            for h in range(H):
                pT = pmisc.tile([128, N], F32, tag="pbig")
                nc.tensor.matmul(pT, Wh_sb[l][h], hT, start=True, stop=True)
                hprojT = sprt.tile([128, N], F32, tag="hprojT")
                nc.scalar.activation(hprojT, pT, AF.Copy)
                pN = pmisc.tile([128, N], F32, tag="pbig")
                for c in range(NCHUNK):
                    nc.tensor.matmul(pN[:, bass.ts(c, 128)], hT_bf[:, bass.ts(c, 128)],
                                     Wh_bf[l][h], start=True, stop=True)
                pn_bf = sproj.tile([128, N], BF16, tag="hprojN")
                nc.vector.tensor_copy(pn_bf, pN)
                hprojN.append(pn_bf)
                zs.append(stage_a(hprojT, ah_sb[l][h], h))
            # ---- stage B: attention bodies ----
            multiT = []
            for h in range(H):
                z_sb, s2cols = zs[h]
                pou_t, rep_sb = stage_b(z_sb, s2cols, hprojN[h], h)
                outT = sbig.tile([128, N], BF16, tag="outT")
                nc.vector.tensor_tensor(outT, pou_t, rep_sb, OP.mult)
                ex = shd.tile([128, N], BF16, tag="elu_ex")
                nc.scalar.activation(ex, outT, AF.Exp)
                nc.vector.tensor_scalar(ex, ex, 1.0, -1.0, OP.min, OP.add)
                mh = smulti.tile([128, N], BF16, tag="multi")
                nc.vector.tensor_tensor(mh, outT, ex, OP.max)
                multiT.append(mh)

            ph2 = pou.tile([128, N], F32, tag="oU")
            for h in range(H):
                nc.tensor.matmul(ph2, Wo_bf[l][:, h, :], multiT[h],
                                 start=(h == 0), stop=(h == H - 1))
            h2T = sbig.tile([128, N], F32, tag="h2T")
            nc.scalar.activation(h2T, ph2, AF.Copy)
            h2N_bf = sbig.tile([128, N], BF16, tag="h2N")
            pn2 = pmisc.tile([128, N], F32, tag="pbig")
            for c in range(NCHUNK):
                nc.tensor.transpose(pn2[:, bass.ts(c, 128)], h2T[:, bass.ts(c, 128)],
                                    ident)
            nc.vector.tensor_copy(h2N_bf, pn2)
            z_sb, s2cols = stage_a(h2T, ao_sb[l], 8, use_bcast=False)
            pou_t, rep_sb = stage_b(z_sb, s2cols, h2N_bf, l, act_chunks=3,
                                    use_bcast=False)
            outsT = sbig.tile([128, N], F32, tag="outT")
            nc.vector.tensor_tensor(outsT, pou_t, rep_sb, OP.mult)

            # ---- residual + LN over partition dim ----
            xs = sbig.tile([128, N], F32, tag="xs")
            nc.vector.tensor_tensor(xs, outsT, residT, OP.add)
            xsq = sbig.tile([128, N], F32, tag="xsq")
            nc.vector.tensor_tensor(xsq, xs, xs, OP.mult)
            pmu = prow.tile([1, N], F32, tag="prow")
            nc.tensor.matmul(pmu, ones_col, xs, start=True, stop=True)
            psq = prow.tile([1, N], F32, tag="prow")
            nc.tensor.matmul(psq, ones_col, xsq, start=True, stop=True)
            mu = srow.tile([1, N], F32, tag="rowL")
            nc.vector.tensor_scalar_mul(mu, pmu, 1.0 / F)
            msq = srow.tile([1, N], F32, tag="rowL")
            nc.vector.tensor_scalar_mul(msq, psq, 1.0 / F)
            mu2 = srow.tile([1, N], F32, tag="rowL")
            nc.vector.tensor_tensor(mu2, mu, mu, OP.mult)
            var = srow.tile([1, N], F32, tag="rowL")
            nc.vector.tensor_tensor(var, msq, mu2, OP.subtract)
            lnv = srow.tile([1, N], F32, tag="rowL")
            nc.scalar.activation(lnv, var, AF.Ln, bias=eps1)
            rstd = srow.tile([1, N], F32, tag="rowL")
            nc.scalar.activation(rstd, lnv, AF.Exp, scale=-0.5)
            mr = srow.tile([1, N], F32, tag="rowL")
            nc.vector.tensor_tensor(mr, mu, rstd, OP.mult)
            r2 = srow.tile([1, N], F32, tag="rowL")
            nc.vector.tensor_scalar_mul(r2, mr, -1.0)
            paff = pmisc.tile([128, N], F32, tag="pbig")
            nc.tensor.matmul(paff, g_row[l], r2, start=True, stop=False)
            nc.tensor.matmul(paff, b_row[l], ones_row, start=False, stop=True)
            prs = pmisc.tile([128, N], F32, tag="pbig")
            nc.tensor.matmul(prs, ones_row[:, 0:128], rstd, start=True, stop=True)
            rep_rstd = sbig.tile([128, N], F32, tag="rep")
            nc.scalar.activation(rep_rstd, prs, AF.Copy)
            y = sbig.tile([128, N], F32, tag="y")
            nc.vector.tensor_tensor(y, xs, rep_rstd, OP.mult)
            nc.vector.tensor_scalar_mul(y, y, g_col[l])
            hT_new = sbig.tile([128, N], F32, tag="hT")
            nc.vector.tensor_tensor(hT_new, y, paff, OP.add)
            if l < L - 1:
                nc.vector.tensor_scalar_max(hT_new, hT_new, 0.0)
            hT = hT_new
            if l < L - 1:
                hT_bf = sbig.tile([128, N], BF16, tag="hTb")
                nc.vector.tensor_copy(hT_bf, hT)

        # ---------------- output: transpose back ----------------
        for c in range(NCHUNK):
            po = pmisc.tile([128, 128], F32, tag="pbig")
            nc.tensor.transpose(po, hT[:, bass.ts(c, 128)], ident)
            osb = shd.tile([128, 128], F32, tag="osb")
            nc.scalar.activation(osb, po, AF.Copy)
            nc.sync.dma_start(out_d[bass.ts(c, 128), :], osb)

    nc.compile()
    return nc


def _get_nc():
    if "nc" not in _CACHE:
        _CACHE["nc"] = build_nc()
    return _CACHE["nc"]


def kernel(**inputs) -> np.ndarray:
    nc = _get_nc()
    shared = {k: np.ascontiguousarray(np.asarray(inputs[k], dtype=np.float32))
              for k in ("Wp", "bp", "W_heads", "a_heads", "W_out", "a_out",
                        "ln_g", "ln_b")}
    x = np.asarray(inputs["x"], dtype=np.float32)
    adj = np.asarray(inputs["adj"], dtype=np.int32)
    in_maps = [dict(x=np.ascontiguousarray(x[b]),
                    adj=np.ascontiguousarray(adj[b]), **shared)
               for b in range(B)]
    res = run_bass_kernel_spmd(nc, in_maps, core_ids=list(range(B)))
    return np.stack([res.results[b]["out"] for b in range(B)])


if __name__ == "__main__":
    rng = np.random.default_rng(0)
    inputs = dict(
        x=rng.normal(size=(B, N, DIN)).astype(np.float32),
        adj=rng.integers(0, 2, size=(B, N, N)).astype(np.int32),
        Wp=(rng.normal(size=(DIN, F)) * 0.12).astype(np.float32),
        bp=np.zeros(F, dtype=np.float32),
        W_heads=(rng.normal(size=(L, H, F, F)) * 0.08).astype(np.float32),
        a_heads=(rng.normal(size=(L, H, 2 * F)) * 0.08).astype(np.float32),
        W_out=(rng.normal(size=(L, H * F, F)) * 0.03).astype(np.float32),
        a_out=(rng.normal(size=(L, 2 * F)) * 0.08).astype(np.float32),
        ln_g=np.ones((L, F), dtype=np.float32),
        ln_b=np.zeros((L, F), dtype=np.float32),
    )
    out = kernel(**inputs)
    print("out", out.shape, out.dtype, np.abs(out).max())



# revision 6
# speedup vs baseline: 1.2174x; 1.2174x over previous
"""GAT spatio-temporal model Trainium2 kernel (v4).

Sharding: data-parallel over batch B=8 -> 8 NeuronCores (1 graph each).

v4 core trick: exp(leaky_relu(s1[n]+s2[m])) = max(E1*E2, E1a*E2a) with
E = exp(s), Ea = exp(alpha*s) (exp monotone, lrelu(x) = max(x, a*x)).
Factor p = E1a[n] * E2[m] * max(E1b[n], E2inv[m]) with E1b = exp((1-a)s1),
E2inv = exp(-(1-a)s2).  E1a[n] is constant along the softmax axis (m) and
cancels; E2[m] folds into the den / AV matmul lhsT weights.  The whole
[N,N] attention tensor is then ONE fused DVE op per 128-chunk:
scalar_tensor_tensor(out, E1b_bcast, E2inv_col, maskT, max, mult).
No N^2 ScalarE work; s1/s2 come from one matmul via precomposed W@a.
All N^2 matmuls bf16; LN matmuls float32r.

Shapes (hardcoded): B=8, N=512, Din=64, H=8, F=128, L=2.
"""
import os
import numpy as np
from contextlib import ExitStack

import concourse.bass as bass
import concourse.tile as tile
from concourse import bacc, mybir
from concourse.bass_utils import run_bass_kernel_spmd
from concourse.masks import make_identity

F32 = mybir.dt.float32
F32R = mybir.dt.float32r
BF16 = mybir.dt.bfloat16
AF = mybir.ActivationFunctionType
OP = mybir.AluOpType

B, N, DIN, H, F, L = 8, 512, 64, 8, 128, 2
NCHUNK = N // 128  # 4
ALPHA = 0.2
BETA = 1.0 - ALPHA
LN_EPS = 1e-5

GP_STT = int(os.environ.get("K_GP_STT", "0"))   # STT chunks on gpsimd
GP_ELU = os.environ.get("K_GP_ELU", "0") == "1"  # ELU tensor_scalar on gpsimd

_CACHE = {}


def _bcast_row(ap_row):
    return bass.AP(tensor=ap_row.tensor, offset=ap_row.offset, ap=[[0, 128], [1, N]])


def _r(ap):
    return ap.bitcast(F32R)


def build_nc():
    nc = bacc.Bacc("TRN2", target_bir_lowering=False, debug=False)

    x_d = nc.dram_tensor("x", [N, DIN], F32, kind="ExternalInput").ap()
    adj_d = nc.dram_tensor("adj", [N, N], mybir.dt.int32, kind="ExternalInput").ap()
    Wp_d = nc.dram_tensor("Wp", [DIN, F], F32, kind="ExternalInput").ap()
    bp_d = nc.dram_tensor("bp", [F], F32, kind="ExternalInput").ap()
    Wh_d = nc.dram_tensor("W_heads", [L, H, F, F], F32, kind="ExternalInput").ap()
    ah_d = nc.dram_tensor("a_heads", [L, H, 2 * F], F32, kind="ExternalInput").ap()
    Wo_d = nc.dram_tensor("W_out", [L, H * F, F], F32, kind="ExternalInput").ap()
    ao_d = nc.dram_tensor("a_out", [L, 2 * F], F32, kind="ExternalInput").ap()
    g_d = nc.dram_tensor("ln_g", [L, F], F32, kind="ExternalInput").ap()
    b_d = nc.dram_tensor("ln_b", [L, F], F32, kind="ExternalInput").ap()
    out_d = nc.dram_tensor("out", [N, F], F32, kind="ExternalOutput").ap()
    # bf16 bounce rows for broadcasts: 8 e1b + 8 rep, reused across layers
    scr_d = [nc.dram_tensor(f"scratch{i}", [1, N], BF16, kind="ExternalOutput").ap()
             for i in range(16)]

    with tile.TileContext(nc) as tc, ExitStack() as ctx:
        const = ctx.enter_context(tc.tile_pool(name="const", bufs=1))
        sx = ctx.enter_context(tc.tile_pool(name="sx", bufs=2))
        sproj = ctx.enter_context(tc.tile_pool(name="sproj", bufs=10))
        sbcast = ctx.enter_context(tc.tile_pool(name="sbcast", bufs=10))
        sexp = ctx.enter_context(tc.tile_pool(name="sexp", bufs=3))
        smulti = ctx.enter_context(tc.tile_pool(name="smulti", bufs=9))
        sbig = ctx.enter_context(tc.tile_pool(name="sbig", bufs=3))
        srow = ctx.enter_context(tc.tile_pool(name="srow", bufs=5))
        shd = ctx.enter_context(tc.tile_pool(name="shd", bufs=4))
        smask = ctx.enter_context(tc.tile_pool(name="smask", bufs=4))
        pou = ctx.enter_context(tc.tile_pool(name="pou", bufs=3, space="PSUM"))
        pmisc = ctx.enter_context(tc.tile_pool(name="pmisc", bufs=2, space="PSUM"))
        prow = ctx.enter_context(tc.tile_pool(name="prow", bufs=2, space="PSUM"))

        # ---------------- constants ----------------
        ones_row = const.tile([1, N], F32)
        nc.vector.memset(ones_row, 1.0)
        ones_row_bf = const.tile([1, N], BF16)
        nc.vector.memset(ones_row_bf, 1.0)
        ones_col = const.tile([128, 1], F32)
        nc.vector.memset(ones_col, 1.0)
        ident = const.tile([128, 128], F32)
        make_identity(nc, ident)
        ident_bf = const.tile([128, 128], BF16)
        nc.vector.tensor_copy(ident_bf, ident)
        eps1 = const.tile([1, 1], F32)
        nc.vector.memset(eps1, LN_EPS)

        Wp_sb = const.tile([DIN, F], F32)
        nc.sync.dma_start(Wp_sb, Wp_d)
        bp_col = const.tile([F, 1], F32)
        nc.sync.dma_start(bp_col, bp_d.rearrange("(f one) -> f one", one=1))
        x_chunks = []
        for c in range(NCHUNK):
            xc = shd.tile([128, DIN], F32, tag="xchunk")
            nc.sync.dma_start(xc, x_d[bass.ts(c, 128), :])
            x_chunks.append(xc)

        # per-layer weight loads spread over DMA queues
        Wh_all = [const.tile([F, H, F], F32, name=f"WhA{l}") for l in range(L)]
        Wh_ball = [const.tile([F, H, F], BF16, name=f"WhB{l}") for l in range(L)]
        nc.sync.dma_start(Wh_all[0], Wh_d[0].rearrange("h i o -> i h o"))
        nc.gpsimd.dma_start(Wh_all[1], Wh_d[1].rearrange("h i o -> i h o"))
        for l in range(L):
            nc.vector.tensor_copy(Wh_ball[l], Wh_all[l])
        Wh_bf = [[Wh_ball[l][:, h, :] for h in range(H)] for l in range(L)]

        ah_all = const.tile([F, L * H, 2], F32)
        nc.sync.dma_start(ah_all, ah_d.rearrange("l h (t f) -> f (l h) t", t=2))
        ah_ball = const.tile([F, L * H, 2], BF16)
        nc.vector.tensor_copy(ah_ball, ah_all)
        ah_bf = [[ah_ball[:, l * H + h, :] for h in range(H)] for l in range(L)]

        wo_f = [const.tile([128, H, F], F32, name=f"WoF{l}") for l in range(L)]
        Wo_ball = [const.tile([128, H, F], BF16, name=f"WoB{l}") for l in range(L)]
        for l in range(L):
            nc.gpsimd.dma_start(wo_f[l], Wo_d[l].rearrange("(c p) f -> p c f", p=128))
            nc.vector.tensor_copy(Wo_ball[l], wo_f[l])
        Wo_bf = Wo_ball

        ao_all = const.tile([F, L, 2], F32)
        nc.sync.dma_start(ao_all, ao_d.rearrange("l (t f) -> f l t", t=2))
        ao_ball = const.tile([F, L, 2], BF16)
        nc.vector.tensor_copy(ao_ball, ao_all)
        ao_bf = [ao_ball[:, l, :] for l in range(L)]

        g_row = [const.tile([1, F], F32, name=f"grow_{l}") for l in range(L)]
        b_row = [const.tile([1, F], F32, name=f"brow_{l}") for l in range(L)]
        g_col = [const.tile([F, 1], F32, name=f"gcol_{l}") for l in range(L)]
        g_row_bf = [const.tile([1, F], BF16, name=f"growb_{l}") for l in range(L)]
        b_row_bf = [const.tile([1, F], BF16, name=f"browb_{l}") for l in range(L)]
        for l in range(L):
            nc.sync.dma_start(g_row[l], g_d[l].rearrange("(one f) -> one f", one=1))
            nc.sync.dma_start(b_row[l], b_d[l].rearrange("(one f) -> one f", one=1))
            nc.sync.dma_start(g_col[l], g_d[l].rearrange("(f one) -> f one", one=1))
            nc.vector.tensor_copy(g_row_bf[l], g_row[l])
            nc.vector.tensor_copy(b_row_bf[l], b_row[l])

        # ------------- WhT (transposed head weights) + Wtilde = W @ a -------
        WhT_ball = [const.tile([F, H, F], BF16, name=f"WhT{l}") for l in range(L)]
        for l in range(L):
            for h in range(H):
                pt = pmisc.tile([128, 128], BF16, tag="pbig")
                nc.tensor.transpose(pt, Wh_bf[l][h], ident_bf)
                nc.scalar.activation(WhT_ball[l][:, h, :], pt, AF.Copy)
        Wt_bf = [const.tile([F, 2 * H], BF16, name=f"Wt{l}") for l in range(L)]
        for l in range(L):
            pw = prow.tile([128, 2 * H], F32, tag="pxc", bufs=1)
            for h in range(H):
                nc.tensor.matmul(pw[:, 2 * h:2 * h + 2], WhT_ball[l][:, h, :],
                                 ah_bf[l][h], start=True, stop=True)
            nc.scalar.activation(Wt_bf[l], pw, AF.Copy)

        # ---------------- x -> xT, input projection ----------------
        xT = const.tile([DIN, N], F32)
        ph = pmisc.tile([128, N], F32, tag="pbig")
        hT = sbig.tile([128, N], F32, tag="hT")
        hT_bf = sbig.tile([128, N], BF16, tag="hTb", bufs=2)
        for c in range(NCHUNK):
            pt = pmisc.tile([DIN, 128], F32, tag="pbig")
            nc.tensor.transpose(pt, x_chunks[c], ident)
            nc.scalar.activation(xT[:, bass.ts(c, 128)], pt, AF.Copy)
            nc.tensor.matmul(ph[:, bass.ts(c, 128)], Wp_sb, xT[:, bass.ts(c, 128)],
                             start=True, stop=True)
            nc.scalar.activation(hT[:, bass.ts(c, 128)], ph[:, bass.ts(c, 128)],
                                 AF.Relu, bias=bp_col)
            nc.vector.tensor_copy(hT_bf[:, bass.ts(c, 128)], hT[:, bass.ts(c, 128)])

        # ---------------- adj -> maskT (bf16, transposed) ----------------
        adj_f = []
        for r in range(NCHUNK):
            ai = shd.tile([128, N], mybir.dt.int32, tag="adji")
            eng = nc.scalar if r % 2 == 0 else nc.sync
            eng.dma_start(ai, adj_d[bass.ts(r, 128), :])
            af = smask.tile([128, N], BF16, tag="adjf")
            nc.vector.tensor_copy(af, ai)
            adj_f.append(af)
        maskT = [const.tile([128, N], BF16, name=f"maskT{c}") for c in range(NCHUNK)]
        for r in range(NCHUNK):
            for c in range(NCHUNK):
                pm = pmisc.tile([128, 128], BF16, tag="pbig")
                nc.tensor.transpose(pm, adj_f[r][:, bass.ts(c, 128)], ident_bf)
                nc.scalar.activation(maskT[c][:, bass.ts(r, 128)], pm, AF.Copy)

        # ------------- attention body (shared by heads & out-att) -----------
        def attention(e1b_sb, e2i_cols, e2_cols, projNp, gp_stt):
            """e1b_sb: [128,N] bf16 bcast of E1b row.  e2i_cols/e2_cols: 4
            [128,1] bf16 col APs (E2inv / E2).  projNp: [128,NCHUNK,128] bf16
            AV lhsT already scaled by E2[m].  Returns (pou_ps, rrow_f32)."""
            s_t = sexp.tile([128, NCHUNK, N], BF16, tag="s_t")
            for c in range(NCHUNK):
                eng = nc.gpsimd if c < gp_stt else nc.vector
                eng.scalar_tensor_tensor(s_t[:, c, :], e1b_sb, e2i_cols[c],
                                         maskT[c], OP.max, OP.mult)
            den_ps = prow.tile([1, N], F32, tag="prow")
            for c in range(NCHUNK):
                nc.tensor.matmul(den_ps, e2_cols[c], s_t[:, c, :],
                                 start=(c == 0), stop=(c == NCHUNK - 1))
            pou_ps = pou.tile([128, N], F32, tag="oU")
            for c in range(NCHUNK):
                nc.tensor.matmul(pou_ps, projNp[:, c, :], s_t[:, c, :],
                                 start=(c == 0), stop=(c == NCHUNK - 1))
            rrow = srow.tile([1, N], F32, tag="rrowf")
            nc.vector.reciprocal_approx_fast(rrow, den_ps)
            return pou_ps, rrow

        # ---------------- layers ----------------
        for l in range(L):
            residT = hT
            # --- rows for all heads: s12[2h] = s1_h, s12[2h+1] = s2_h
            s12_ps = prow.tile([2 * H, N], F32, tag="prow")
            nc.tensor.matmul(s12_ps, Wt_bf[l], hT_bf, start=True, stop=True)
            Eblk = sx.tile([16, N], BF16, tag="Eblk")   # exp(+beta*s): rows 2h = E1b
            nc.scalar.activation(Eblk, s12_ps, AF.Exp, scale=BETA)
            Xneg = sx.tile([16, N], BF16, tag="Xneg")   # exp(-beta*s): 2h+1 = E2inv
            nc.scalar.activation(Xneg, s12_ps, AF.Exp, scale=-BETA)
            Xpos = sx.tile([16, N], BF16, tag="Xpos")   # exp(s): 2h+1 = E2
            nc.scalar.activation(Xpos, s12_ps, AF.Exp, scale=1.0)
            # launch E1b broadcasts early (DRAM bounce)
            e1b = []
            for h in range(H):
                nc.sync.dma_start(scr_d[h], Eblk[2 * h:2 * h + 1, :])
                eb = sbcast.tile([128, N], BF16, tag="e1b")
                nc.sync.dma_start(eb, _bcast_row(scr_d[h][0, :]))
                e1b.append(eb)
            # --- columns: transpose Xneg/Xpos -> Xcols [128, 8*16]
            xc_ps = prow.tile([128, 8 * 16], BF16, tag="pxc", bufs=1)
            for c in range(NCHUNK):
                nc.tensor.transpose(xc_ps[:, c * 16:(c + 1) * 16],
                                    Xneg[:, bass.ts(c, 128)], ident_bf[0:16, 0:16])
                nc.tensor.transpose(xc_ps[:, 64 + c * 16:64 + (c + 1) * 16],
                                    Xpos[:, bass.ts(c, 128)], ident_bf[0:16, 0:16])
            Xcols = sx.tile([128, 8 * 16], F32, tag="Xcols")
            nc.scalar.activation(Xcols, xc_ps, AF.Copy)
            Xcols_bf = sx.tile([128, 8 * 16], BF16, tag="Xcolsb")
            nc.vector.tensor_copy(Xcols_bf, xc_ps)

            def e2i_col(h, c):
                j = c * 16 + 2 * h + 1
                return Xcols[:, j:j + 1]

            def e2_col(h, c):
                j = 64 + c * 16 + 2 * h + 1
                return Xcols[:, j:j + 1]

            def e2_col_bf(h, c):
                j = 64 + c * 16 + 2 * h + 1
                return Xcols_bf[:, j:j + 1]

            # --- projN per head (scaled by E2[m])
            projNp = []
            for h in range(H):
                pN = pmisc.tile([128, N], F32, tag="pbig")
                for c in range(NCHUNK):
                    nc.tensor.matmul(pN[:, bass.ts(c, 128)], hT_bf[:, bass.ts(c, 128)],
                                     Wh_bf[l][h], start=True, stop=True)
                pn_bf = shd.tile([128, N], BF16, tag="pnbf")
                nc.scalar.activation(pn_bf, pN, AF.Copy)
                pp = sproj.tile([128, NCHUNK, 128], BF16, tag="projNp")
                for c in range(NCHUNK):
                    nc.vector.tensor_scalar_mul(pp[:, c, :], pn_bf[:, bass.ts(c, 128)],
                                                e2_col(h, c))
                projNp.append(pp)
            # --- attention per head + ELU
            multiT = []
            for h in range(H):
                pou_ps, rrow = attention(
                    e1b[h],
                    [e2i_col(h, c) for c in range(NCHUNK)],
                    [e2_col_bf(h, c) for c in range(NCHUNK)],
                    projNp[h], GP_STT)
                rrow_bf = srow.tile([1, N], BF16, tag="rrowb")
                nc.vector.tensor_copy(rrow_bf, rrow)
                nc.sync.dma_start(scr_d[8 + h], rrow_bf)
                rep = sbcast.tile([128, N], BF16, tag="rep", bufs=4)
                nc.sync.dma_start(rep, _bcast_row(scr_d[8 + h][0, :]))
                pou_bf = shd.tile([128, N], BF16, tag="poubf")
                nc.scalar.activation(pou_bf, pou_ps, AF.Copy)
                outT = sbig.tile([128, N], BF16, tag="outT", bufs=4)
                nc.vector.tensor_tensor(outT, pou_bf, rep, OP.mult)
                ex = shd.tile([128, N], BF16, tag="elu_ex")
                nc.scalar.activation(ex, outT, AF.Exp)
                eng = nc.gpsimd if GP_ELU else nc.vector
                eng.tensor_scalar(ex, ex, 1.0, -1.0, OP.min, OP.add)
                mh = smulti.tile([128, N], BF16, tag="multi")
                nc.vector.tensor_tensor(mh, outT, ex, OP.max)
                multiT.append(mh)

            # --- W_out projection
            ph2 = pou.tile([128, N], F32, tag="oU")
            for h in range(H):
                nc.tensor.matmul(ph2, Wo_bf[l][:, h, :], multiT[h],
                                 start=(h == 0), stop=(h == H - 1))
            h2_bf = sbig.tile([128, N], BF16, tag="h2b", bufs=2)
            nc.scalar.activation(h2_bf, ph2, AF.Copy)

            # --- single out-attention
            s12o_ps = prow.tile([2, N], F32, tag="prow")
            nc.tensor.matmul(s12o_ps, ao_bf[l], h2_bf, start=True, stop=True)
            Xo_b = sx.tile([2, N], BF16, tag="Xo_b")    # row 0 = E1b_o
            nc.scalar.activation(Xo_b, s12o_ps, AF.Exp, scale=BETA)
            Xo_nb = sx.tile([2, N], BF16, tag="Xo_nb")  # row 1 = E2inv_o
            nc.scalar.activation(Xo_nb, s12o_ps, AF.Exp, scale=-BETA)
            Xo_1 = sx.tile([2, N], BF16, tag="Xo_1")    # row 1 = E2_o
            nc.scalar.activation(Xo_1, s12o_ps, AF.Exp, scale=1.0)
            xo_ps = prow.tile([128, 16], BF16, tag="pxc", bufs=1)
            for c in range(NCHUNK):
                nc.tensor.transpose(xo_ps[:, c * 2:(c + 1) * 2],
                                    Xo_nb[:, bass.ts(c, 128)], ident_bf[0:2, 0:2])
                nc.tensor.transpose(xo_ps[:, 8 + c * 2:8 + (c + 1) * 2],
                                    Xo_1[:, bass.ts(c, 128)], ident_bf[0:2, 0:2])
            Xoc = sx.tile([128, 16], F32, tag="Xoc")
            nc.scalar.activation(Xoc, xo_ps, AF.Copy)
            Xoc_bf = sx.tile([128, 16], BF16, tag="Xocb")
            nc.vector.tensor_copy(Xoc_bf, xo_ps)
            # E1b_o broadcast via PE rank-1 (low latency; PE idle here)
            ebo_ps = pmisc.tile([128, N], F32, tag="pbig")
            nc.tensor.matmul(ebo_ps, ones_row_bf[:, 0:128], Xo_b[0:1, :], start=True, stop=True)
            e1bo = sbcast.tile([128, N], BF16, tag="e1b")
            nc.scalar.activation(e1bo, ebo_ps, AF.Copy)
            # h2N via transposes, scaled by E2o[m]
            h2n_ps = pmisc.tile([128, N], BF16, tag="pbig")
            for c in range(NCHUNK):
                nc.tensor.transpose(h2n_ps[:, bass.ts(c, 128)],
                                    h2_bf[:, bass.ts(c, 128)], ident_bf)
            h2n_bf = shd.tile([128, N], BF16, tag="pnbf")
            nc.scalar.activation(h2n_bf, h2n_ps, AF.Copy)
            h2Np = sproj.tile([128, NCHUNK, 128], BF16, tag="projNp")
            for c in range(NCHUNK):
                nc.vector.tensor_scalar_mul(h2Np[:, c, :], h2n_bf[:, bass.ts(c, 128)],
                                            Xoc[:, 8 + c * 2 + 1:8 + c * 2 + 2])
            pouo_ps, rro = attention(
                e1bo,
                [Xoc[:, c * 2 + 1:c * 2 + 2] for c in range(NCHUNK)],
                [Xoc_bf[:, 8 + c * 2 + 1:8 + c * 2 + 2] for c in range(NCHUNK)],
                h2Np, 0)
            # rep via PE rank-1 (bf16 row as rhs)
            rro_bf = srow.tile([1, N], BF16, tag="rrowb")
            nc.vector.tensor_copy(rro_bf, rro)
            repo_ps = pmisc.tile([128, N], F32, tag="pbig")
            nc.tensor.matmul(repo_ps, ones_row_bf[:, 0:128], rro_bf, start=True, stop=True)
            repo = sbig.tile([128, N], F32, tag="repo", bufs=2)
            nc.scalar.activation(repo, repo_ps, AF.Copy)
            pouo_sb = sbig.tile([128, N], F32, tag="pouo", bufs=2)
            nc.scalar.activation(pouo_sb, pouo_ps, AF.Copy)
            outsT = sbig.tile([128, N], F32, tag="outsT", bufs=2)
            nc.vector.tensor_tensor(outsT, pouo_sb, repo, OP.mult)

            # ---- residual + LN over partition dim ----
            xs = sbig.tile([128, N], F32, tag="xs", bufs=2)
            nc.vector.tensor_tensor(xs, outsT, residT, OP.add)
            xsq = sbig.tile([128, N], F32, tag="xsq", bufs=2)
            nc.vector.tensor_tensor(xsq, xs, xs, OP.mult)
            pmu = prow.tile([1, N], F32, tag="prow")
            nc.tensor.matmul(pmu, ones_col, xs, start=True, stop=True)
            psq = prow.tile([1, N], F32, tag="prow")
            nc.tensor.matmul(psq, ones_col, xsq, start=True, stop=True)
            mu = srow.tile([1, N], F32, tag="rowL")
            nc.vector.tensor_scalar_mul(mu, pmu, 1.0 / F)
            msq = srow.tile([1, N], F32, tag="rowL")
            nc.vector.tensor_scalar_mul(msq, psq, 1.0 / F)
            mu2 = srow.tile([1, N], F32, tag="rowL")
            nc.vector.tensor_tensor(mu2, mu, mu, OP.mult)
            var = srow.tile([1, N], F32, tag="rowL")
            nc.vector.tensor_tensor(var, msq, mu2, OP.subtract)
            lnv = srow.tile([1, N], F32, tag="rowL")
            nc.scalar.activation(lnv, var, AF.Ln, bias=eps1)
            rstd = srow.tile([1, N], F32, tag="rowL")
            nc.scalar.activation(rstd, lnv, AF.Exp, scale=-0.5)
            mr = srow.tile([1, N], F32, tag="rowL")
            nc.vector.tensor_tensor(mr, mu, rstd, OP.mult)
            r2 = srow.tile([1, N], BF16, tag="rowLb")
            nc.vector.tensor_scalar_mul(r2, mr, -1.0)
            rstd_bf = srow.tile([1, N], BF16, tag="rowLb")
            nc.vector.tensor_copy(rstd_bf, rstd)
            paff = pmisc.tile([128, N], F32, tag="pbig")
            nc.tensor.matmul(paff, g_row_bf[l], r2, start=True, stop=False)
            nc.tensor.matmul(paff, b_row_bf[l], ones_row_bf, start=False, stop=True)
            prs = pmisc.tile([128, N], F32, tag="pbig")
            nc.tensor.matmul(prs, ones_row_bf[:, 0:128], rstd_bf,
                             start=True, stop=True)
            rep_rstd = sbig.tile([128, N], F32, tag="repo", bufs=2)
            nc.scalar.activation(rep_rstd, prs, AF.Copy)
            y = sbig.tile([128, N], F32, tag="y", bufs=2)
            nc.vector.tensor_tensor(y, xs, rep_rstd, OP.mult)
            nc.vector.tensor_scalar_mul(y, y, g_col[l])
            hT_new = sbig.tile([128, N], F32, tag="hT")
            nc.vector.tensor_tensor(hT_new, y, paff, OP.add)
            if l < L - 1:
                nc.vector.tensor_scalar_max(hT_new, hT_new, 0.0)
            hT = hT_new
            if l < L - 1:
                hT_bf = sbig.tile([128, N], BF16, tag="hTb", bufs=2)
                nc.vector.tensor_copy(hT_bf, hT)

        # ---------------- output: transpose back ----------------
        for c in range(NCHUNK):
            po = pmisc.tile([128, 128], F32, tag="pbig")
            nc.tensor.transpose(po, hT[:, bass.ts(c, 128)], ident)
            osb = shd.tile([128, 128], F32, tag="osb")
            nc.scalar.activation(osb, po, AF.Copy)
            nc.sync.dma_start(out_d[bass.ts(c, 128), :], osb)

    nc.compile()
    return nc


def _get_nc():
    if "nc" not in _CACHE:
        _CACHE["nc"] = build_nc()
    return _CACHE["nc"]


def kernel(**inputs) -> np.ndarray:
    nc = _get_nc()
    shared = {k: np.ascontiguousarray(np.asarray(inputs[k], dtype=np.float32))
              for k in ("Wp", "bp", "W_heads", "a_heads", "W_out", "a_out",
                        "ln_g", "ln_b")}
    x = np.asarray(inputs["x"], dtype=np.float32)
    adj = np.asarray(inputs["adj"], dtype=np.int32)
    in_maps = [dict(x=np.ascontiguousarray(x[b]),
                    adj=np.ascontiguousarray(adj[b]), **shared)
               for b in range(B)]
    res = run_bass_kernel_spmd(nc, in_maps, core_ids=list(range(B)))
    return np.stack([res.results[b]["out"] for b in range(B)])


if __name__ == "__main__":
    rng = np.random.default_rng(0)
    inputs = dict(
        x=rng.normal(size=(B, N, DIN)).astype(np.float32),
        adj=rng.integers(0, 2, size=(B, N, N)).astype(np.int32),
        Wp=(rng.normal(size=(DIN, F)) * 0.12).astype(np.float32),
        bp=np.zeros(F, dtype=np.float32),
        W_heads=(rng.normal(size=(L, H, F, F)) * 0.08).astype(np.float32),
        a_heads=(rng.normal(size=(L, H, 2 * F)) * 0.08).astype(np.float32),
        W_out=(rng.normal(size=(L, H * F, F)) * 0.03).astype(np.float32),
        a_out=(rng.normal(size=(L, 2 * F)) * 0.08).astype(np.float32),
        ln_g=np.ones((L, F), dtype=np.float32),
        ln_b=np.zeros((L, F), dtype=np.float32),
    )
    out = kernel(**inputs)
    print("out", out.shape, out.dtype, np.abs(out).max())


# revision 11
# speedup vs baseline: 1.2732x; 1.0459x over previous
"""GAT spatio-temporal model Trainium2 kernel (v4).

Sharding: data-parallel over batch B=8 -> 8 NeuronCores (1 graph each).

v4 core trick: exp(leaky_relu(s1[n]+s2[m])) = max(E1*E2, E1a*E2a) with
E = exp(s), Ea = exp(alpha*s) (exp monotone, lrelu(x) = max(x, a*x)).
Factor p = E1a[n] * E2[m] * max(E1b[n], E2inv[m]) with E1b = exp((1-a)s1),
E2inv = exp(-(1-a)s2).  E1a[n] is constant along the softmax axis (m) and
cancels; E2[m] folds into the den / AV matmul lhsT weights.  The whole
[N,N] attention tensor is then ONE fused DVE op per 128-chunk:
scalar_tensor_tensor(out, E1b_bcast, E2inv_col, maskT, max, mult).
No N^2 ScalarE work; s1/s2 come from one matmul via precomposed W@a.
All N^2 matmuls bf16; LN matmuls float32r.

Shapes (hardcoded): B=8, N=512, Din=64, H=8, F=128, L=2.
"""
import os
import numpy as np
from contextlib import ExitStack

import concourse.bass as bass
import concourse.tile as tile
from concourse import bacc, mybir
from concourse.bass_utils import run_bass_kernel_spmd
from concourse.masks import make_identity

F32 = mybir.dt.float32
F32R = mybir.dt.float32r
BF16 = mybir.dt.bfloat16
AF = mybir.ActivationFunctionType
OP = mybir.AluOpType

B, N, DIN, H, F, L = 8, 512, 64, 8, 128, 2
NCHUNK = N // 128  # 4
ALPHA = 0.2
BETA = 1.0 - ALPHA
LN_EPS = 1e-5

GP_STT = int(os.environ.get("K_GP_STT", "0"))   # STT chunks on gpsimd
GP_ELU = os.environ.get("K_GP_ELU", "0") == "1"  # ELU tensor_scalar on gpsimd
GP_EMAX = os.environ.get("K_GP_EMAX", "0") == "1"  # ELU max on gpsimd
GP_PSC = int(os.environ.get("K_GP_PSC", "0"))   # projNp scale chunks on gpsimd

_CACHE = {}


def _bcast_row(ap_row):
    return bass.AP(tensor=ap_row.tensor, offset=ap_row.offset, ap=[[0, 128], [1, N]])


def _r(ap):
    return ap.bitcast(F32R)


def build_nc():
    nc = bacc.Bacc("TRN2", target_bir_lowering=False, debug=False)

    x_d = nc.dram_tensor("x", [N, DIN], F32, kind="ExternalInput").ap()
    adj_d = nc.dram_tensor("adj", [N, N], mybir.dt.int32, kind="ExternalInput").ap()
    Wp_d = nc.dram_tensor("Wp", [DIN, F], F32, kind="ExternalInput").ap()
    bp_d = nc.dram_tensor("bp", [F], F32, kind="ExternalInput").ap()
    Wh_d = nc.dram_tensor("W_heads", [L, H, F, F], F32, kind="ExternalInput").ap()
    ah_d = nc.dram_tensor("a_heads", [L, H, 2 * F], F32, kind="ExternalInput").ap()
    Wo_d = nc.dram_tensor("W_out", [L, H * F, F], F32, kind="ExternalInput").ap()
    ao_d = nc.dram_tensor("a_out", [L, 2 * F], F32, kind="ExternalInput").ap()
    g_d = nc.dram_tensor("ln_g", [L, F], F32, kind="ExternalInput").ap()
    b_d = nc.dram_tensor("ln_b", [L, F], F32, kind="ExternalInput").ap()
    out_d = nc.dram_tensor("out", [N, F], F32, kind="ExternalOutput").ap()
    # DRAM bounce buffer for E1b row broadcasts (one per layer, 2 slots)
    ebl_d = [nc.dram_tensor(f"eblk{l}", [16, N], BF16, kind="ExternalOutput").ap()
             for l in range(L)]

    with tile.TileContext(nc) as tc, ExitStack() as ctx:
        const = ctx.enter_context(tc.tile_pool(name="const", bufs=1))
        sx = ctx.enter_context(tc.tile_pool(name="sx", bufs=2))
        sproj = ctx.enter_context(tc.tile_pool(name="sproj", bufs=10))
        sbcast = ctx.enter_context(tc.tile_pool(name="sbcast", bufs=10))
        sexp = ctx.enter_context(tc.tile_pool(name="sexp", bufs=3))
        smulti = ctx.enter_context(tc.tile_pool(name="smulti", bufs=9))
        sbig = ctx.enter_context(tc.tile_pool(name="sbig", bufs=3))
        srow = ctx.enter_context(tc.tile_pool(name="srow", bufs=5))
        shd = ctx.enter_context(tc.tile_pool(name="shd", bufs=4))
        smask = ctx.enter_context(tc.tile_pool(name="smask", bufs=4))
        pou = ctx.enter_context(tc.tile_pool(name="pou", bufs=3, space="PSUM"))
        pmisc = ctx.enter_context(tc.tile_pool(name="pmisc", bufs=2, space="PSUM"))
        prow = ctx.enter_context(tc.tile_pool(name="prow", bufs=3, space="PSUM"))

        # ---------------- constants ----------------
        ones_row = const.tile([1, N], F32)
        nc.vector.memset(ones_row, 1.0)
        ones_row_bf = const.tile([1, N], BF16)
        nc.vector.memset(ones_row_bf, 1.0)
        ones_col = const.tile([128, 1], F32)
        nc.vector.memset(ones_col, 1.0)
        ident = const.tile([128, 128], F32)
        make_identity(nc, ident)
        ident_bf = const.tile([128, 128], BF16)
        nc.vector.tensor_copy(ident_bf, ident)
        eps1 = const.tile([1, 1], F32)
        nc.vector.memset(eps1, LN_EPS)

        Wp_sb = const.tile([DIN, F], F32)
        nc.sync.dma_start(Wp_sb, Wp_d)
        bp_col = const.tile([F, 1], F32)
        nc.sync.dma_start(bp_col, bp_d.rearrange("(f one) -> f one", one=1))
        x_chunks = []
        for c in range(NCHUNK):
            xc = shd.tile([128, DIN], F32, tag="xchunk")
            nc.sync.dma_start(xc, x_d[bass.ts(c, 128), :])
            x_chunks.append(xc)

        # per-layer weight loads spread over DMA queues
        Wh_all = [const.tile([F, H, F], F32, name=f"WhA{l}") for l in range(L)]
        Wh_ball = [const.tile([F, H, F], BF16, name=f"WhB{l}") for l in range(L)]
        nc.sync.dma_start(Wh_all[0], Wh_d[0].rearrange("h i o -> i h o"))
        nc.gpsimd.dma_start(Wh_all[1], Wh_d[1].rearrange("h i o -> i h o"))
        for l in range(L):
            nc.vector.tensor_copy(Wh_ball[l], Wh_all[l])
        Wh_bf = [[Wh_ball[l][:, h, :] for h in range(H)] for l in range(L)]

        ah_all = const.tile([F, L * H, 2], F32)
        nc.sync.dma_start(ah_all, ah_d.rearrange("l h (t f) -> f (l h) t", t=2))
        ah_ball = const.tile([F, L * H, 2], BF16)
        nc.vector.tensor_copy(ah_ball, ah_all)
        ah_bf = [[ah_ball[:, l * H + h, :] for h in range(H)] for l in range(L)]

        wo_f = [const.tile([128, H, F], F32, name=f"WoF{l}") for l in range(L)]
        Wo_ball = [const.tile([128, H, F], BF16, name=f"WoB{l}") for l in range(L)]
        for l in range(L):
            nc.gpsimd.dma_start(wo_f[l], Wo_d[l].rearrange("(c p) f -> p c f", p=128))
            nc.vector.tensor_copy(Wo_ball[l], wo_f[l])
        Wo_bf = Wo_ball

        ao_all = const.tile([F, L, 2], F32)
        nc.sync.dma_start(ao_all, ao_d.rearrange("l (t f) -> f l t", t=2))
        ao_ball = const.tile([F, L, 2], BF16)
        nc.vector.tensor_copy(ao_ball, ao_all)
        ao_bf = [ao_ball[:, l, :] for l in range(L)]

        g_row = [const.tile([1, F], F32, name=f"grow_{l}") for l in range(L)]
        b_row = [const.tile([1, F], F32, name=f"brow_{l}") for l in range(L)]
        g_col = [const.tile([F, 1], F32, name=f"gcol_{l}") for l in range(L)]
        g_row_bf = [const.tile([1, F], BF16, name=f"growb_{l}") for l in range(L)]
        b_row_bf = [const.tile([1, F], BF16, name=f"browb_{l}") for l in range(L)]
        for l in range(L):
            nc.sync.dma_start(g_row[l], g_d[l].rearrange("(one f) -> one f", one=1))
            nc.sync.dma_start(b_row[l], b_d[l].rearrange("(one f) -> one f", one=1))
            nc.sync.dma_start(g_col[l], g_d[l].rearrange("(f one) -> f one", one=1))
            nc.vector.tensor_copy(g_row_bf[l], g_row[l])
            nc.vector.tensor_copy(b_row_bf[l], b_row[l])

        # ------------- WhT (transposed head weights) + Wtilde = W @ a -------
        WhT_ball = [const.tile([F, H, F], BF16, name=f"WhT{l}") for l in range(L)]
        for l in range(L):
            for h in range(H):
                pt = pmisc.tile([128, 128], BF16, tag="pbig")
                nc.tensor.transpose(pt, Wh_bf[l][h], ident_bf)
                nc.scalar.activation(WhT_ball[l][:, h, :], pt, AF.Copy)
        Wt_bf = [const.tile([F, 2 * H], BF16, name=f"Wt{l}") for l in range(L)]
        for l in range(L):
            pw = prow.tile([128, 2 * H], F32, tag="prow")
            for h in range(H):
                nc.tensor.matmul(pw[:, 2 * h:2 * h + 2], WhT_ball[l][:, h, :],
                                 ah_bf[l][h], start=True, stop=True)
            nc.scalar.activation(Wt_bf[l], pw, AF.Copy)

        # ---------------- x -> xT, input projection ----------------
        xT = const.tile([DIN, N], F32)
        ph = pmisc.tile([128, N], F32, tag="pbig")
        hT = sbig.tile([128, N], F32, tag="hT")
        hT_bf = sbig.tile([128, N], BF16, tag="hTb", bufs=2)
        for c in range(NCHUNK):
            pt = pmisc.tile([DIN, 128], F32, tag="pbig")
            nc.tensor.transpose(pt, x_chunks[c], ident)
            nc.scalar.activation(xT[:, bass.ts(c, 128)], pt, AF.Copy)
            nc.tensor.matmul(ph[:, bass.ts(c, 128)], Wp_sb, xT[:, bass.ts(c, 128)],
                             start=True, stop=True)
            nc.scalar.activation(hT[:, bass.ts(c, 128)], ph[:, bass.ts(c, 128)],
                                 AF.Relu, bias=bp_col)
            nc.vector.tensor_copy(hT_bf[:, bass.ts(c, 128)], hT[:, bass.ts(c, 128)])

        # ---------------- adj -> maskT (bf16, transposed) ----------------
        adj_f = []
        for r in range(NCHUNK):
            ai = shd.tile([128, N], mybir.dt.int32, tag="adji")
            eng = nc.scalar if r % 2 == 0 else nc.sync
            eng.dma_start(ai, adj_d[bass.ts(r, 128), :])
            af = smask.tile([128, N], BF16, tag="adjf")
            nc.vector.tensor_copy(af, ai)
            adj_f.append(af)
        maskT = [const.tile([128, N], BF16, name=f"maskT{c}") for c in range(NCHUNK)]
        for r in range(NCHUNK):
            for c in range(NCHUNK):
                pm = pmisc.tile([128, 128], BF16, tag="pbig")
                nc.tensor.transpose(pm, adj_f[r][:, bass.ts(c, 128)], ident_bf)
                nc.scalar.activation(maskT[c][:, bass.ts(r, 128)], pm, AF.Copy)

        # ------------- attention body (shared by heads & out-att) -----------
        def attention(e1b_sb, e2i_cols, e2_cols, projNp, gp_stt, out_f32=False):
            """e1b_sb: [128,N] bf16 bcast of E1b row.  e2i_cols/e2_cols: 4
            [128,1] col APs (E2inv f32 / E2 bf16).  projNp: [128,NCHUNK,128]
            bf16 AV lhsT already scaled by E2[m].  Returns outT = pou/den."""
            s_t = sexp.tile([128, NCHUNK, N], BF16, tag="s_t")
            for c in range(NCHUNK):
                eng = nc.gpsimd if c < gp_stt else nc.vector
                eng.scalar_tensor_tensor(s_t[:, c, :], e1b_sb, e2i_cols[c],
                                         maskT[c], OP.max, OP.mult)
            den_ps = prow.tile([1, N], F32, tag="prow")
            for c in range(NCHUNK):
                nc.tensor.matmul(den_ps, e2_cols[c], s_t[:, c, :],
                                 start=(c == 0), stop=(c == NCHUNK - 1))
            pou_ps = pou.tile([128, N], F32, tag="oU")
            for c in range(NCHUNK):
                nc.tensor.matmul(pou_ps, projNp[:, c, :], s_t[:, c, :],
                                 start=(c == 0), stop=(c == NCHUNK - 1))
            rrow = srow.tile([1, N], F32, tag="rrowf")
            nc.vector.reciprocal_approx_fast(rrow, den_ps)
            rrow_bf = srow.tile([1, N], BF16, tag="rrowb")
            nc.vector.tensor_copy(rrow_bf, rrow)
            rep_ps = pou.tile([128, N], F32, tag="oU")
            nc.tensor.matmul(rep_ps, ones_row_bf[:, 0:128], rrow_bf,
                             start=True, stop=True)
            rep = sbcast.tile([128, N], BF16, tag="rep", bufs=4)
            nc.scalar.activation(rep, rep_ps, AF.Copy)
            outT = sbig.tile([128, N], F32 if out_f32 else BF16, tag="outT",
                             bufs=4)
            nc.vector.tensor_tensor(outT, pou_ps, rep, OP.mult)
            return outT

        # ---------------- layers ----------------
        for l in range(L):
            residT = hT
            # --- rows for all heads: s12[2h] = s1_h, s12[2h+1] = s2_h
            s12_ps = prow.tile([2 * H, N], F32, tag="prow")
            nc.tensor.matmul(s12_ps, Wt_bf[l], hT_bf, start=True, stop=True)
            Eblk = sx.tile([16, N], BF16, tag="Eblk")   # exp(+beta*s): rows 2h = E1b
            nc.scalar.activation(Eblk, s12_ps, AF.Exp, scale=BETA)
            Xneg = sx.tile([16, N], BF16, tag="Xneg")   # exp(-beta*s): 2h+1 = E2inv
            nc.scalar.activation(Xneg, s12_ps, AF.Exp, scale=-BETA)
            Xpos = sx.tile([16, N], BF16, tag="Xpos")   # exp(s): 2h+1 = E2
            nc.scalar.activation(Xpos, s12_ps, AF.Exp, scale=1.0)
            # E1b broadcasts: one DRAM bounce write of all rows, then one
            # stride-0 broadcast read per head, spread across DMA queues
            dmaq = [nc.sync, nc.scalar, nc.gpsimd]
            nc.sync.dma_start(ebl_d[l], Eblk)
            e1b = []
            for h in range(H):
                row = ebl_d[l][2 * h, :]
                src_bc = bass.AP(tensor=row.tensor, offset=row.offset,
                                 ap=[[0, 128], [1, N]])
                eb = sbcast.tile([128, N], BF16, tag="e1b")
                dmaq[h % 3].dma_start(eb, src_bc)
                e1b.append(eb)
            # --- columns: transpose Xneg/Xpos -> Xcols [128, 8*16]
            xc_ps = prow.tile([128, 8 * 16], BF16, tag="prow")
            for c in range(NCHUNK):
                nc.tensor.transpose(xc_ps[:, c * 16:(c + 1) * 16],
                                    Xneg[:, bass.ts(c, 128)], ident_bf[0:16, 0:16])
                nc.tensor.transpose(xc_ps[:, 64 + c * 16:64 + (c + 1) * 16],
                                    Xpos[:, bass.ts(c, 128)], ident_bf[0:16, 0:16])
            Xcols = sx.tile([128, 8 * 16], F32, tag="Xcols")
            nc.scalar.activation(Xcols, xc_ps, AF.Copy)
            Xcols_bf = sx.tile([128, 8 * 16], BF16, tag="Xcolsb")
            nc.vector.tensor_copy(Xcols_bf, xc_ps)

            def e2i_col(h, c):
                j = c * 16 + 2 * h + 1
                return Xcols[:, j:j + 1]

            def e2_col(h, c):
                j = 64 + c * 16 + 2 * h + 1
                return Xcols[:, j:j + 1]

            def e2_col_bf(h, c):
                j = 64 + c * 16 + 2 * h + 1
                return Xcols_bf[:, j:j + 1]

            # --- projN per head (scaled by E2[m])
            projNp = []
            for h in range(H):
                pN = pmisc.tile([128, N], F32, tag="pbig")
                for c in range(NCHUNK):
                    nc.tensor.matmul(pN[:, bass.ts(c, 128)], hT_bf[:, bass.ts(c, 128)],
                                     Wh_bf[l][h], start=True, stop=True)
                pn_bf = shd.tile([128, N], BF16, tag="pnbf")
                nc.scalar.activation(pn_bf, pN, AF.Copy)
                pp = sproj.tile([128, NCHUNK, 128], BF16, tag="projNp")
                for c in range(NCHUNK):
                    eng = nc.gpsimd if c < GP_PSC else nc.vector
                    eng.tensor_scalar_mul(pp[:, c, :], pn_bf[:, bass.ts(c, 128)],
                                          e2_col(h, c))
                projNp.append(pp)
            # --- attention per head + ELU
            multiT = []
            for h in range(H):
                outT = attention(
                    e1b[h],
                    [e2i_col(h, c) for c in range(NCHUNK)],
                    [e2_col_bf(h, c) for c in range(NCHUNK)],
                    projNp[h], GP_STT)
                ex = shd.tile([128, N], BF16, tag="elu_ex")
                nc.scalar.activation(ex, outT, AF.Exp)
                eng = nc.gpsimd if GP_ELU else nc.vector
                eng.tensor_scalar(ex, ex, 1.0, -1.0, OP.min, OP.add)
                mh = smulti.tile([128, N], BF16, tag="multi")
                eng2 = nc.gpsimd if GP_EMAX else nc.vector
                eng2.tensor_tensor(mh, outT, ex, OP.max)
                multiT.append(mh)

            # --- W_out projection
            ph2 = pou.tile([128, N], F32, tag="oU")
            for h in range(H):
                nc.tensor.matmul(ph2, Wo_bf[l][:, h, :], multiT[h],
                                 start=(h == 0), stop=(h == H - 1))
            h2_bf = sbig.tile([128, N], BF16, tag="h2b", bufs=2)
            nc.scalar.activation(h2_bf, ph2, AF.Copy)

            # --- single out-attention
            s12o_ps = prow.tile([2, N], F32, tag="prow")
            nc.tensor.matmul(s12o_ps, ao_bf[l], h2_bf, start=True, stop=True)
            Xo_b = sx.tile([2, N], BF16, tag="Xo_b")    # row 0 = E1b_o
            nc.scalar.activation(Xo_b, s12o_ps, AF.Exp, scale=BETA)
            Xo_nb = sx.tile([2, N], BF16, tag="Xo_nb")  # row 1 = E2inv_o
            nc.scalar.activation(Xo_nb, s12o_ps, AF.Exp, scale=-BETA)
            Xo_1 = sx.tile([2, N], BF16, tag="Xo_1")    # row 1 = E2_o
            nc.scalar.activation(Xo_1, s12o_ps, AF.Exp, scale=1.0)
            xo_ps = prow.tile([128, 16], BF16, tag="prow")
            for c in range(NCHUNK):
                nc.tensor.transpose(xo_ps[:, c * 2:(c + 1) * 2],
                                    Xo_nb[:, bass.ts(c, 128)], ident_bf[0:2, 0:2])
                nc.tensor.transpose(xo_ps[:, 8 + c * 2:8 + (c + 1) * 2],
                                    Xo_1[:, bass.ts(c, 128)], ident_bf[0:2, 0:2])
            Xoc = sx.tile([128, 16], F32, tag="Xoc")
            nc.scalar.activation(Xoc, xo_ps, AF.Copy)
            Xoc_bf = sx.tile([128, 16], BF16, tag="Xocb")
            nc.vector.tensor_copy(Xoc_bf, xo_ps)
            # E1b_o broadcast via PE rank-1 (low latency; PE idle here)
            ebo_ps = pmisc.tile([128, N], F32, tag="pbig")
            nc.tensor.matmul(ebo_ps, ones_row_bf[:, 0:128], Xo_b[0:1, :], start=True, stop=True)
            e1bo = sbcast.tile([128, N], BF16, tag="e1b")
            nc.scalar.activation(e1bo, ebo_ps, AF.Copy)
            # h2N via transposes, scaled by E2o[m]
            h2n_ps = pmisc.tile([128, N], BF16, tag="pbig")
            for c in range(NCHUNK):
                nc.tensor.transpose(h2n_ps[:, bass.ts(c, 128)],
                                    h2_bf[:, bass.ts(c, 128)], ident_bf)
            h2n_bf = shd.tile([128, N], BF16, tag="pnbf")
            nc.scalar.activation(h2n_bf, h2n_ps, AF.Copy)
            h2Np = sproj.tile([128, NCHUNK, 128], BF16, tag="projNp")
            for c in range(NCHUNK):
                nc.vector.tensor_scalar_mul(h2Np[:, c, :], h2n_bf[:, bass.ts(c, 128)],
                                            Xoc[:, 8 + c * 2 + 1:8 + c * 2 + 2])
            outsT = attention(
                e1bo,
                [Xoc[:, c * 2 + 1:c * 2 + 2] for c in range(NCHUNK)],
                [Xoc_bf[:, 8 + c * 2 + 1:8 + c * 2 + 2] for c in range(NCHUNK)],
                h2Np, 0, out_f32=True)

            # ---- residual + LN over partition dim ----
            xs = sbig.tile([128, N], F32, tag="xs", bufs=2)
            nc.vector.tensor_tensor(xs, outsT, residT, OP.add)
            xsq = sbig.tile([128, N], F32, tag="xsq", bufs=2)
            nc.vector.tensor_tensor(xsq, xs, xs, OP.mult)
            pmu = prow.tile([1, N], F32, tag="prow")
            nc.tensor.matmul(pmu, ones_col, xs, start=True, stop=True)
            psq = prow.tile([1, N], F32, tag="prow")
            nc.tensor.matmul(psq, ones_col, xsq, start=True, stop=True)
            mu = srow.tile([1, N], F32, tag="rowL")
            nc.vector.tensor_scalar_mul(mu, pmu, 1.0 / F)
            msq = srow.tile([1, N], F32, tag="rowL")
            nc.vector.tensor_scalar_mul(msq, psq, 1.0 / F)
            mu2 = srow.tile([1, N], F32, tag="rowL")
            nc.vector.tensor_tensor(mu2, mu, mu, OP.mult)
            var = srow.tile([1, N], F32, tag="rowL")
            nc.vector.tensor_tensor(var, msq, mu2, OP.subtract)
            lnv = srow.tile([1, N], F32, tag="rowL")
            nc.scalar.activation(lnv, var, AF.Ln, bias=eps1)
            rstd = srow.tile([1, N], F32, tag="rowL")
            nc.scalar.activation(rstd, lnv, AF.Exp, scale=-0.5)
            mr = srow.tile([1, N], F32, tag="rowL")
            nc.vector.tensor_tensor(mr, mu, rstd, OP.mult)
            r2 = srow.tile([1, N], BF16, tag="rowLb")
            nc.vector.tensor_scalar_mul(r2, mr, -1.0)
            rstd_bf = srow.tile([1, N], BF16, tag="rowLb")
            nc.vector.tensor_copy(rstd_bf, rstd)
            paff = pmisc.tile([128, N], F32, tag="pbig")
            nc.tensor.matmul(paff, g_row_bf[l], r2, start=True, stop=False)
            nc.tensor.matmul(paff, b_row_bf[l], ones_row_bf, start=False, stop=True)
            prs = pmisc.tile([128, N], F32, tag="pbig")
            nc.tensor.matmul(prs, ones_row_bf[:, 0:128], rstd_bf,
                             start=True, stop=True)
            rep_rstd = sbig.tile([128, N], F32, tag="repo", bufs=2)
            nc.scalar.activation(rep_rstd, prs, AF.Copy)
            y = sbig.tile([128, N], F32, tag="y", bufs=2)
            nc.vector.tensor_tensor(y, xs, rep_rstd, OP.mult)
            nc.vector.tensor_scalar_mul(y, y, g_col[l])
            hT_new = sbig.tile([128, N], F32, tag="hT")
            nc.vector.tensor_tensor(hT_new, y, paff, OP.add)
            if l < L - 1:
                nc.vector.tensor_scalar_max(hT_new, hT_new, 0.0)
            hT = hT_new
            if l < L - 1:
                hT_bf = sbig.tile([128, N], BF16, tag="hTb", bufs=2)
                nc.vector.tensor_copy(hT_bf, hT)

        # ---------------- output: transpose back ----------------
        for c in range(NCHUNK):
            po = pmisc.tile([128, 128], F32, tag="pbig")
            nc.tensor.transpose(po, hT[:, bass.ts(c, 128)], ident)
            osb = shd.tile([128, 128], F32, tag="osb")
            nc.scalar.activation(osb, po, AF.Copy)
            nc.sync.dma_start(out_d[bass.ts(c, 128), :], osb)

    nc.compile()
    return nc


def _get_nc():
    if "nc" not in _CACHE:
        _CACHE["nc"] = build_nc()
    return _CACHE["nc"]


def kernel(**inputs) -> np.ndarray:
    nc = _get_nc()
    shared = {k: np.ascontiguousarray(np.asarray(inputs[k], dtype=np.float32))
              for k in ("Wp", "bp", "W_heads", "a_heads", "W_out", "a_out",
                        "ln_g", "ln_b")}
    x = np.asarray(inputs["x"], dtype=np.float32)
    adj = np.asarray(inputs["adj"], dtype=np.int32)
    in_maps = [dict(x=np.ascontiguousarray(x[b]),
                    adj=np.ascontiguousarray(adj[b]), **shared)
               for b in range(B)]
    res = run_bass_kernel_spmd(nc, in_maps, core_ids=list(range(B)))
    return np.stack([res.results[b]["out"] for b in range(B)])


if __name__ == "__main__":
    rng = np.random.default_rng(0)
    inputs = dict(
        x=rng.normal(size=(B, N, DIN)).astype(np.float32),
        adj=rng.integers(0, 2, size=(B, N, N)).astype(np.int32),
        Wp=(rng.normal(size=(DIN, F)) * 0.12).astype(np.float32),
        bp=np.zeros(F, dtype=np.float32),
        W_heads=(rng.normal(size=(L, H, F, F)) * 0.08).astype(np.float32),
        a_heads=(rng.normal(size=(L, H, 2 * F)) * 0.08).astype(np.float32),
        W_out=(rng.normal(size=(L, H * F, F)) * 0.03).astype(np.float32),
        a_out=(rng.normal(size=(L, 2 * F)) * 0.08).astype(np.float32),
        ln_g=np.ones((L, F), dtype=np.float32),
        ln_b=np.zeros((L, F), dtype=np.float32),
    )
    out = kernel(**inputs)
    print("out", out.shape, out.dtype, np.abs(out).max())


# revision 12
# speedup vs baseline: 1.3344x; 1.0480x over previous
"""GAT spatio-temporal model Trainium2 kernel (v4).

Sharding: data-parallel over batch B=8 -> 8 NeuronCores (1 graph each).

v4 core trick: exp(leaky_relu(s1[n]+s2[m])) = max(E1*E2, E1a*E2a) with
E = exp(s), Ea = exp(alpha*s) (exp monotone, lrelu(x) = max(x, a*x)).
Factor p = E1a[n] * E2[m] * max(E1b[n], E2inv[m]) with E1b = exp((1-a)s1),
E2inv = exp(-(1-a)s2).  E1a[n] is constant along the softmax axis (m) and
cancels; E2[m] folds into the den / AV matmul lhsT weights.  The whole
[N,N] attention tensor is then ONE fused DVE op per 128-chunk:
scalar_tensor_tensor(out, E1b_bcast, E2inv_col, maskT, max, mult).
No N^2 ScalarE work; s1/s2 come from one matmul via precomposed W@a.
All N^2 matmuls bf16; LN matmuls float32r.

Shapes (hardcoded): B=8, N=512, Din=64, H=8, F=128, L=2.
"""
import os
import numpy as np
from contextlib import ExitStack

import concourse.bass as bass
import concourse.tile as tile
from concourse import bacc, mybir
from concourse.bass_utils import run_bass_kernel_spmd
from concourse.masks import make_identity

F32 = mybir.dt.float32
F32R = mybir.dt.float32r
BF16 = mybir.dt.bfloat16
AF = mybir.ActivationFunctionType
OP = mybir.AluOpType

B, N, DIN, H, F, L = 8, 512, 64, 8, 128, 2
NCHUNK = N // 128  # 4
ALPHA = 0.2
BETA = 1.0 - ALPHA
LN_EPS = 1e-5

GP_STT = int(os.environ.get("K_GP_STT", "0"))   # STT chunks on gpsimd
GP_ELU = os.environ.get("K_GP_ELU", "0") == "1"  # ELU tensor_scalar on gpsimd
GP_EMAX = os.environ.get("K_GP_EMAX", "0") == "1"  # ELU max on gpsimd
GP_PSC = int(os.environ.get("K_GP_PSC", "0"))   # projNp scale chunks on gpsimd

_CACHE = {}


def _bcast_row(ap_row):
    return bass.AP(tensor=ap_row.tensor, offset=ap_row.offset, ap=[[0, 128], [1, N]])


def _r(ap):
    return ap.bitcast(F32R)


def build_nc():
    nc = bacc.Bacc("TRN2", target_bir_lowering=False, debug=False)

    x_d = nc.dram_tensor("x", [N, DIN], F32, kind="ExternalInput").ap()
    adj_d = nc.dram_tensor("adj", [N, N], mybir.dt.int32, kind="ExternalInput").ap()
    Wp_d = nc.dram_tensor("Wp", [DIN, F], F32, kind="ExternalInput").ap()
    bp_d = nc.dram_tensor("bp", [F], F32, kind="ExternalInput").ap()
    Wh_d = nc.dram_tensor("W_heads", [L, H, F, F], F32, kind="ExternalInput").ap()
    ah_d = nc.dram_tensor("a_heads", [L, H, 2 * F], F32, kind="ExternalInput").ap()
    Wo_d = nc.dram_tensor("W_out", [L, H * F, F], F32, kind="ExternalInput").ap()
    ao_d = nc.dram_tensor("a_out", [L, 2 * F], F32, kind="ExternalInput").ap()
    g_d = nc.dram_tensor("ln_g", [L, F], F32, kind="ExternalInput").ap()
    b_d = nc.dram_tensor("ln_b", [L, F], F32, kind="ExternalInput").ap()
    out_d = nc.dram_tensor("out", [N, F], F32, kind="ExternalOutput").ap()
    # DRAM bounce buffer for E1b row broadcasts (one per layer, 2 slots)
    ebl_d = [nc.dram_tensor(f"eblk{l}", [16, N], BF16, kind="ExternalOutput").ap()
             for l in range(L)]

    with tile.TileContext(nc) as tc, ExitStack() as ctx:
        const = ctx.enter_context(tc.tile_pool(name="const", bufs=1))
        sx = ctx.enter_context(tc.tile_pool(name="sx", bufs=2))
        sproj = ctx.enter_context(tc.tile_pool(name="sproj", bufs=10))
        sbcast = ctx.enter_context(tc.tile_pool(name="sbcast", bufs=10))
        sexp = ctx.enter_context(tc.tile_pool(name="sexp", bufs=4))
        smulti = ctx.enter_context(tc.tile_pool(name="smulti", bufs=9))
        sbig = ctx.enter_context(tc.tile_pool(name="sbig", bufs=3))
        srow = ctx.enter_context(tc.tile_pool(name="srow", bufs=5))
        shd = ctx.enter_context(tc.tile_pool(name="shd", bufs=4))
        smask = ctx.enter_context(tc.tile_pool(name="smask", bufs=4))
        pou = ctx.enter_context(tc.tile_pool(name="pou", bufs=4, space="PSUM"))
        pmisc = ctx.enter_context(tc.tile_pool(name="pmisc", bufs=2, space="PSUM"))
        prow = ctx.enter_context(tc.tile_pool(name="prow", bufs=2, space="PSUM"))

        # ---------------- constants ----------------
        ones_row = const.tile([1, N], F32)
        nc.vector.memset(ones_row, 1.0)
        ones_row_bf = const.tile([1, N], BF16)
        nc.vector.memset(ones_row_bf, 1.0)
        ones_col = const.tile([128, 1], F32)
        nc.vector.memset(ones_col, 1.0)
        ident = const.tile([128, 128], F32)
        make_identity(nc, ident)
        ident_bf = const.tile([128, 128], BF16)
        nc.vector.tensor_copy(ident_bf, ident)
        eps1 = const.tile([1, 1], F32)
        nc.vector.memset(eps1, LN_EPS)

        Wp_sb = const.tile([DIN, F], F32)
        nc.sync.dma_start(Wp_sb, Wp_d)
        bp_col = const.tile([F, 1], F32)
        nc.sync.dma_start(bp_col, bp_d.rearrange("(f one) -> f one", one=1))
        x_chunks = []
        for c in range(NCHUNK):
            xc = shd.tile([128, DIN], F32, tag="xchunk")
            nc.sync.dma_start(xc, x_d[bass.ts(c, 128), :])
            x_chunks.append(xc)

        # per-layer weight loads spread over DMA queues
        Wh_all = [const.tile([F, H, F], F32, name=f"WhA{l}") for l in range(L)]
        Wh_ball = [const.tile([F, H, F], BF16, name=f"WhB{l}") for l in range(L)]
        nc.sync.dma_start(Wh_all[0], Wh_d[0].rearrange("h i o -> i h o"))
        nc.gpsimd.dma_start(Wh_all[1], Wh_d[1].rearrange("h i o -> i h o"))
        for l in range(L):
            nc.vector.tensor_copy(Wh_ball[l], Wh_all[l])
        Wh_bf = [[Wh_ball[l][:, h, :] for h in range(H)] for l in range(L)]

        ah_all = const.tile([F, L * H, 2], F32)
        nc.sync.dma_start(ah_all, ah_d.rearrange("l h (t f) -> f (l h) t", t=2))
        ah_ball = const.tile([F, L * H, 2], BF16)
        nc.vector.tensor_copy(ah_ball, ah_all)
        ah_bf = [[ah_ball[:, l * H + h, :] for h in range(H)] for l in range(L)]

        wo_f = [const.tile([128, H, F], F32, name=f"WoF{l}") for l in range(L)]
        Wo_ball = [const.tile([128, H, F], BF16, name=f"WoB{l}") for l in range(L)]
        for l in range(L):
            nc.gpsimd.dma_start(wo_f[l], Wo_d[l].rearrange("(c p) f -> p c f", p=128))
            nc.vector.tensor_copy(Wo_ball[l], wo_f[l])
        Wo_bf = Wo_ball

        ao_all = const.tile([F, L, 2], F32)
        nc.sync.dma_start(ao_all, ao_d.rearrange("l (t f) -> f l t", t=2))
        ao_ball = const.tile([F, L, 2], BF16)
        nc.vector.tensor_copy(ao_ball, ao_all)
        ao_bf = [ao_ball[:, l, :] for l in range(L)]

        g_all = const.tile([1, L, F], F32)
        nc.scalar.dma_start(g_all, g_d.rearrange("l f -> (l f)").rearrange(
            "(one l f) -> one l f", one=1, l=L))
        b_all = const.tile([1, L, F], F32)
        nc.scalar.dma_start(b_all, b_d.rearrange("l f -> (l f)").rearrange(
            "(one l f) -> one l f", one=1, l=L))
        gc_all = const.tile([F, L], F32)
        nc.scalar.dma_start(gc_all, g_d.rearrange("l f -> f l"))
        g_row = [g_all[:, l, :] for l in range(L)]
        b_row = [b_all[:, l, :] for l in range(L)]
        g_col = [gc_all[:, l:l + 1] for l in range(L)]
        gb_bf = const.tile([1, 2 * L, F], BF16)
        nc.vector.tensor_copy(gb_bf[:, 0:L, :], g_all)
        nc.vector.tensor_copy(gb_bf[:, L:2 * L, :], b_all)
        g_row_bf = [gb_bf[:, l, :] for l in range(L)]
        b_row_bf = [gb_bf[:, L + l, :] for l in range(L)]

        # ------------- WhT (transposed head weights) + Wtilde = W @ a -------
        WhT_ball = [const.tile([F, H, F], BF16, name=f"WhT{l}") for l in range(L)]
        for l in range(L):
            for h in range(H):
                pt = pmisc.tile([128, 128], BF16, tag="pbig")
                nc.tensor.transpose(pt, Wh_bf[l][h], ident_bf)
                nc.scalar.activation(WhT_ball[l][:, h, :], pt, AF.Copy)
        Wt_bf = [const.tile([F, 2 * H], BF16, name=f"Wt{l}") for l in range(L)]
        for l in range(L):
            pw = prow.tile([128, 2 * H], F32, tag="prow")
            for h in range(H):
                nc.tensor.matmul(pw[:, 2 * h:2 * h + 2], WhT_ball[l][:, h, :],
                                 ah_bf[l][h], start=True, stop=True)
            nc.scalar.activation(Wt_bf[l], pw, AF.Copy)

        # ---------------- x -> xT, input projection ----------------
        xT = const.tile([DIN, N], F32)
        ph = pmisc.tile([128, N], F32, tag="pbig")
        hT = sbig.tile([128, N], F32, tag="hT")
        hT_bf = sbig.tile([128, N], BF16, tag="hTb", bufs=2)
        for c in range(NCHUNK):
            pt = pmisc.tile([DIN, 128], F32, tag="pbig")
            nc.tensor.transpose(pt, x_chunks[c], ident)
            nc.scalar.activation(xT[:, bass.ts(c, 128)], pt, AF.Copy)
            nc.tensor.matmul(ph[:, bass.ts(c, 128)], Wp_sb, xT[:, bass.ts(c, 128)],
                             start=True, stop=True)
            nc.scalar.activation(hT[:, bass.ts(c, 128)], ph[:, bass.ts(c, 128)],
                                 AF.Relu, bias=bp_col)
            nc.vector.tensor_copy(hT_bf[:, bass.ts(c, 128)], hT[:, bass.ts(c, 128)])

        # ---------------- adj -> maskT (bf16, transposed) ----------------
        adj_f = []
        for r in range(NCHUNK):
            ai = shd.tile([128, N], mybir.dt.int32, tag="adji")
            eng = nc.scalar if r % 2 == 0 else nc.sync
            eng.dma_start(ai, adj_d[bass.ts(r, 128), :])
            af = smask.tile([128, N], BF16, tag="adjf")
            nc.vector.tensor_copy(af, ai)
            adj_f.append(af)
        maskT = [const.tile([128, N], BF16, name=f"maskT{c}") for c in range(NCHUNK)]
        for r in range(NCHUNK):
            for c in range(NCHUNK):
                pm = pmisc.tile([128, 128], BF16, tag="pbig")
                nc.tensor.transpose(pm, adj_f[r][:, bass.ts(c, 128)], ident_bf)
                nc.scalar.activation(maskT[c][:, bass.ts(r, 128)], pm, AF.Copy)

        # ------------- attention body (shared by heads & out-att) -----------
        def attention(e1b_sb, e2i_cols, e2_cols, projNp, gp_stt, out_f32=False):
            """e1b_sb: [128,N] bf16 bcast of E1b row.  e2i_cols/e2_cols: 4
            [128,1] col APs (E2inv f32 / E2 bf16).  projNp: [128,NCHUNK,128]
            bf16 AV lhsT already scaled by E2[m].  Returns outT = pou/den."""
            s_t = sexp.tile([128, NCHUNK, N], BF16, tag="s_t")
            for c in range(NCHUNK):
                eng = nc.gpsimd if c < gp_stt else nc.vector
                eng.scalar_tensor_tensor(s_t[:, c, :], e1b_sb, e2i_cols[c],
                                         maskT[c], OP.max, OP.mult)
            den_ps = prow.tile([1, N], F32, tag="prow")
            for c in range(NCHUNK):
                nc.tensor.matmul(den_ps, e2_cols[c], s_t[:, c, :],
                                 start=(c == 0), stop=(c == NCHUNK - 1))
            pou_ps = pou.tile([128, N], F32, tag="oU")
            for c in range(NCHUNK):
                nc.tensor.matmul(pou_ps, projNp[:, c, :], s_t[:, c, :],
                                 start=(c == 0), stop=(c == NCHUNK - 1))
            rrow = srow.tile([1, N], F32, tag="rrowf")
            nc.vector.reciprocal_approx_fast(rrow, den_ps)
            rrow_bf = srow.tile([1, N], BF16, tag="rrowb")
            nc.vector.tensor_copy(rrow_bf, rrow)
            rep_ps = pou.tile([128, N], F32, tag="oU")
            nc.tensor.matmul(rep_ps, ones_row_bf[:, 0:128], rrow_bf,
                             start=True, stop=True)
            rep = sbcast.tile([128, N], BF16, tag="rep", bufs=6)
            nc.scalar.activation(rep, rep_ps, AF.Copy)
            outT = sbig.tile([128, N], F32 if out_f32 else BF16, tag="outT",
                             bufs=4)
            nc.vector.tensor_tensor(outT, pou_ps, rep, OP.mult)
            return outT

        # ---------------- layers ----------------
        for l in range(L):
            residT = hT
            # --- rows for all heads: s12[2h] = s1_h, s12[2h+1] = s2_h
            s12_ps = prow.tile([2 * H, N], F32, tag="prow")
            nc.tensor.matmul(s12_ps, Wt_bf[l], hT_bf, start=True, stop=True)
            Eblk = sx.tile([16, N], BF16, tag="Eblk")   # exp(+beta*s): rows 2h = E1b
            nc.scalar.activation(Eblk, s12_ps, AF.Exp, scale=BETA)
            Xneg = sx.tile([16, N], BF16, tag="Xneg")   # exp(-beta*s): 2h+1 = E2inv
            nc.scalar.activation(Xneg, s12_ps, AF.Exp, scale=-BETA)
            Xpos = sx.tile([16, N], BF16, tag="Xpos")   # exp(s): 2h+1 = E2
            nc.scalar.activation(Xpos, s12_ps, AF.Exp, scale=1.0)
            # E1b broadcasts: one DRAM bounce write of all rows, then one
            # stride-0 broadcast read per head, spread across DMA queues
            dmaq = [nc.sync, nc.scalar, nc.gpsimd]
            nc.sync.dma_start(ebl_d[l], Eblk)
            e1b = []
            for h in range(H):
                row = ebl_d[l][2 * h, :]
                src_bc = bass.AP(tensor=row.tensor, offset=row.offset,
                                 ap=[[0, 128], [1, N]])
                eb = sbcast.tile([128, N], BF16, tag="e1b")
                dmaq[h % 3].dma_start(eb, src_bc)
                e1b.append(eb)
            # --- columns: transpose Xneg/Xpos -> Xcols [128, 8*16]
            xc_ps = prow.tile([128, 8 * 16], BF16, tag="prow")
            for c in range(NCHUNK):
                nc.tensor.transpose(xc_ps[:, c * 16:(c + 1) * 16],
                                    Xneg[:, bass.ts(c, 128)], ident_bf[0:16, 0:16])
                nc.tensor.transpose(xc_ps[:, 64 + c * 16:64 + (c + 1) * 16],
                                    Xpos[:, bass.ts(c, 128)], ident_bf[0:16, 0:16])
            Xcols = sx.tile([128, 8 * 16], F32, tag="Xcols")
            nc.scalar.activation(Xcols, xc_ps, AF.Copy)
            Xcols_bf = sx.tile([128, 8 * 16], BF16, tag="Xcolsb")
            nc.vector.tensor_copy(Xcols_bf, xc_ps)

            def e2i_col(h, c):
                j = c * 16 + 2 * h + 1
                return Xcols[:, j:j + 1]

            def e2_col(h, c):
                j = 64 + c * 16 + 2 * h + 1
                return Xcols[:, j:j + 1]

            def e2_col_bf(h, c):
                j = 64 + c * 16 + 2 * h + 1
                return Xcols_bf[:, j:j + 1]

            # --- projN per head (scaled by E2[m])
            projNp = []
            for h in range(H):
                pN = pmisc.tile([128, N], F32, tag="pbig")
                for c in range(NCHUNK):
                    nc.tensor.matmul(pN[:, bass.ts(c, 128)], hT_bf[:, bass.ts(c, 128)],
                                     Wh_bf[l][h], start=True, stop=True)
                pp = sproj.tile([128, NCHUNK, 128], BF16, tag="projNp")
                for c in range(NCHUNK):
                    nc.scalar.activation(pp[:, c, :], pN[:, bass.ts(c, 128)],
                                         AF.Identity, scale=e2_col(h, c))
                projNp.append(pp)
            # --- attention per head + ELU
            multiT = []
            for h in range(H):
                outT = attention(
                    e1b[h],
                    [e2i_col(h, c) for c in range(NCHUNK)],
                    [e2_col_bf(h, c) for c in range(NCHUNK)],
                    projNp[h], GP_STT)
                ex = shd.tile([128, N], BF16, tag="elu_ex")
                nc.scalar.activation(ex, outT, AF.Exp)
                eng = nc.gpsimd if GP_ELU else nc.vector
                eng.tensor_scalar(ex, ex, 1.0, -1.0, OP.min, OP.add)
                mh = smulti.tile([128, N], BF16, tag="multi")
                eng2 = nc.gpsimd if GP_EMAX else nc.vector
                eng2.tensor_tensor(mh, outT, ex, OP.max)
                multiT.append(mh)

            # --- W_out projection
            ph2 = pou.tile([128, N], F32, tag="oU")
            for h in range(H):
                nc.tensor.matmul(ph2, Wo_bf[l][:, h, :], multiT[h],
                                 start=(h == 0), stop=(h == H - 1))
            h2_bf = sbig.tile([128, N], BF16, tag="h2b", bufs=2)
            nc.scalar.activation(h2_bf, ph2, AF.Copy)

            # --- single out-attention
            s12o_ps = prow.tile([2, N], F32, tag="prow")
            nc.tensor.matmul(s12o_ps, ao_bf[l], h2_bf, start=True, stop=True)
            Xo_b = sx.tile([2, N], BF16, tag="Xo_b")    # row 0 = E1b_o
            nc.scalar.activation(Xo_b, s12o_ps, AF.Exp, scale=BETA)
            Xo_nb = sx.tile([2, N], BF16, tag="Xo_nb")  # row 1 = E2inv_o
            nc.scalar.activation(Xo_nb, s12o_ps, AF.Exp, scale=-BETA)
            Xo_1 = sx.tile([2, N], BF16, tag="Xo_1")    # row 1 = E2_o
            nc.scalar.activation(Xo_1, s12o_ps, AF.Exp, scale=1.0)
            xo_ps = prow.tile([128, 16], BF16, tag="prow")
            for c in range(NCHUNK):
                nc.tensor.transpose(xo_ps[:, c * 2:(c + 1) * 2],
                                    Xo_nb[:, bass.ts(c, 128)], ident_bf[0:2, 0:2])
                nc.tensor.transpose(xo_ps[:, 8 + c * 2:8 + (c + 1) * 2],
                                    Xo_1[:, bass.ts(c, 128)], ident_bf[0:2, 0:2])
            Xoc = sx.tile([128, 16], F32, tag="Xoc")
            nc.scalar.activation(Xoc, xo_ps, AF.Copy)
            Xoc_bf = sx.tile([128, 16], BF16, tag="Xocb")
            nc.vector.tensor_copy(Xoc_bf, xo_ps)
            # E1b_o broadcast via PE rank-1 (low latency; PE idle here)
            ebo_ps = pmisc.tile([128, N], F32, tag="pbig")
            nc.tensor.matmul(ebo_ps, ones_row_bf[:, 0:128], Xo_b[0:1, :], start=True, stop=True)
            e1bo = sbcast.tile([128, N], BF16, tag="e1b")
            nc.scalar.activation(e1bo, ebo_ps, AF.Copy)
            # h2N via transposes, scaled by E2o[m]
            h2n_ps = pmisc.tile([128, N], BF16, tag="pbig")
            for c in range(NCHUNK):
                nc.tensor.transpose(h2n_ps[:, bass.ts(c, 128)],
                                    h2_bf[:, bass.ts(c, 128)], ident_bf)
            h2Np = sproj.tile([128, NCHUNK, 128], BF16, tag="projNp")
            for c in range(NCHUNK):
                nc.scalar.activation(h2Np[:, c, :], h2n_ps[:, bass.ts(c, 128)],
                                     AF.Identity, scale=Xoc[:, 8 + c * 2 + 1:8 + c * 2 + 2])
            outsT = attention(
                e1bo,
                [Xoc[:, c * 2 + 1:c * 2 + 2] for c in range(NCHUNK)],
                [Xoc_bf[:, 8 + c * 2 + 1:8 + c * 2 + 2] for c in range(NCHUNK)],
                h2Np, 0, out_f32=True)

            # ---- residual + LN over partition dim ----
            xs = sbig.tile([128, N], F32, tag="xs", bufs=2)
            nc.vector.tensor_tensor(xs, outsT, residT, OP.add)
            xsq = sbig.tile([128, N], F32, tag="xsq", bufs=2)
            nc.scalar.activation(xsq, xs, AF.Square)
            pmu = prow.tile([1, N], F32, tag="prow")
            nc.tensor.matmul(pmu, ones_col, xs, start=True, stop=True)
            psq = prow.tile([1, N], F32, tag="prow")
            nc.tensor.matmul(psq, ones_col, xsq, start=True, stop=True)
            mu = srow.tile([1, N], F32, tag="rowL")
            nc.vector.tensor_scalar_mul(mu, pmu, 1.0 / F)
            msq = srow.tile([1, N], F32, tag="rowL")
            nc.vector.tensor_scalar_mul(msq, psq, 1.0 / F)
            mu2 = srow.tile([1, N], F32, tag="rowL")
            nc.vector.tensor_tensor(mu2, mu, mu, OP.mult)
            var = srow.tile([1, N], F32, tag="rowL")
            nc.vector.tensor_tensor(var, msq, mu2, OP.subtract)
            lnv = srow.tile([1, N], F32, tag="rowL")
            nc.scalar.activation(lnv, var, AF.Ln, bias=eps1)
            rstd = srow.tile([1, N], F32, tag="rowL")
            nc.scalar.activation(rstd, lnv, AF.Exp, scale=-0.5)
            mr = srow.tile([1, N], F32, tag="rowL")
            nc.vector.tensor_tensor(mr, mu, rstd, OP.mult)
            r2 = srow.tile([1, N], BF16, tag="rowLb")
            nc.vector.tensor_scalar_mul(r2, mr, -1.0)
            rstd_bf = srow.tile([1, N], BF16, tag="rowLb")
            nc.vector.tensor_copy(rstd_bf, rstd)
            paff = pmisc.tile([128, N], F32, tag="pbig")
            nc.tensor.matmul(paff, g_row_bf[l], r2, start=True, stop=False)
            nc.tensor.matmul(paff, b_row_bf[l], ones_row_bf, start=False, stop=True)
            prs = pmisc.tile([128, N], F32, tag="pbig")
            nc.tensor.matmul(prs, g_row_bf[l], rstd_bf,
                             start=True, stop=True)
            rep_grstd = sbig.tile([128, N], F32, tag="repo", bufs=2)
            nc.scalar.activation(rep_grstd, prs, AF.Copy)
            y = sbig.tile([128, N], F32, tag="y", bufs=2)
            nc.vector.tensor_tensor(y, xs, rep_grstd, OP.mult)
            hT_new = sbig.tile([128, N], F32, tag="hT")
            nc.vector.tensor_tensor(hT_new, y, paff, OP.add)
            if l < L - 1:
                nc.vector.tensor_scalar_max(hT_new, hT_new, 0.0)
            hT = hT_new
            if l < L - 1:
                hT_bf = sbig.tile([128, N], BF16, tag="hTb", bufs=2)
                nc.vector.tensor_copy(hT_bf, hT)

        # ---------------- output: transpose back ----------------
        for c in range(NCHUNK):
            po = pmisc.tile([128, 128], F32, tag="pbig")
            nc.tensor.transpose(po, hT[:, bass.ts(c, 128)], ident)
            osb = shd.tile([128, 128], F32, tag="osb")
            nc.scalar.activation(osb, po, AF.Copy)
            nc.sync.dma_start(out_d[bass.ts(c, 128), :], osb)

    nc.compile()
    return nc


def _get_nc():
    if "nc" not in _CACHE:
        _CACHE["nc"] = build_nc()
    return _CACHE["nc"]


def kernel(**inputs) -> np.ndarray:
    nc = _get_nc()
    shared = {k: np.ascontiguousarray(np.asarray(inputs[k], dtype=np.float32))
              for k in ("Wp", "bp", "W_heads", "a_heads", "W_out", "a_out",
                        "ln_g", "ln_b")}
    x = np.asarray(inputs["x"], dtype=np.float32)
    adj = np.asarray(inputs["adj"], dtype=np.int32)
    in_maps = [dict(x=np.ascontiguousarray(x[b]),
                    adj=np.ascontiguousarray(adj[b]), **shared)
               for b in range(B)]
    res = run_bass_kernel_spmd(nc, in_maps, core_ids=list(range(B)))
    return np.stack([res.results[b]["out"] for b in range(B)])


if __name__ == "__main__":
    rng = np.random.default_rng(0)
    inputs = dict(
        x=rng.normal(size=(B, N, DIN)).astype(np.float32),
        adj=rng.integers(0, 2, size=(B, N, N)).astype(np.int32),
        Wp=(rng.normal(size=(DIN, F)) * 0.12).astype(np.float32),
        bp=np.zeros(F, dtype=np.float32),
        W_heads=(rng.normal(size=(L, H, F, F)) * 0.08).astype(np.float32),
        a_heads=(rng.normal(size=(L, H, 2 * F)) * 0.08).astype(np.float32),
        W_out=(rng.normal(size=(L, H * F, F)) * 0.03).astype(np.float32),
        a_out=(rng.normal(size=(L, 2 * F)) * 0.08).astype(np.float32),
        ln_g=np.ones((L, F), dtype=np.float32),
        ln_b=np.zeros((L, F), dtype=np.float32),
    )
    out = kernel(**inputs)
    print("out", out.shape, out.dtype, np.abs(out).max())


# revision 14
# speedup vs baseline: 1.4436x; 1.0818x over previous
"""GAT spatio-temporal model Trainium2 kernel (v4).

Sharding: data-parallel over batch B=8 -> 8 NeuronCores (1 graph each).

v4 core trick: exp(leaky_relu(s1[n]+s2[m])) = max(E1*E2, E1a*E2a) with
E = exp(s), Ea = exp(alpha*s) (exp monotone, lrelu(x) = max(x, a*x)).
Factor p = E1a[n] * E2[m] * max(E1b[n], E2inv[m]) with E1b = exp((1-a)s1),
E2inv = exp(-(1-a)s2).  E1a[n] is constant along the softmax axis (m) and
cancels; E2[m] folds into the den / AV matmul lhsT weights.  The whole
[N,N] attention tensor is then ONE fused DVE op per 128-chunk:
scalar_tensor_tensor(out, E1b_bcast, E2inv_col, maskT, max, mult).
No N^2 ScalarE work; s1/s2 come from one matmul via precomposed W@a.
All N^2 matmuls bf16; LN matmuls float32r.

Shapes (hardcoded): B=8, N=512, Din=64, H=8, F=128, L=2.
"""
import os
import numpy as np
from contextlib import ExitStack

import concourse.bass as bass
import concourse.tile as tile
from concourse import bacc, mybir
from concourse.bass_utils import run_bass_kernel_spmd
from concourse.masks import make_identity

F32 = mybir.dt.float32
F32R = mybir.dt.float32r
BF16 = mybir.dt.bfloat16
AF = mybir.ActivationFunctionType
OP = mybir.AluOpType

B, N, DIN, H, F, L = 8, 512, 64, 8, 128, 2
NCHUNK = N // 128  # 4
ALPHA = 0.2
BETA = 1.0 - ALPHA
LN_EPS = 1e-5

GP_STT = int(os.environ.get("K_GP_STT", "0"))   # STT chunks on gpsimd
GP_ELU = os.environ.get("K_GP_ELU", "0") == "1"  # ELU tensor_scalar on gpsimd
GP_EMAX = os.environ.get("K_GP_EMAX", "0") == "1"  # ELU max on gpsimd
GP_PSC = int(os.environ.get("K_GP_PSC", "0"))   # projNp scale chunks on gpsimd

_CACHE = {}


def _bcast_row(ap_row):
    return bass.AP(tensor=ap_row.tensor, offset=ap_row.offset, ap=[[0, 128], [1, N]])


def _r(ap):
    return ap.bitcast(F32R)


def build_nc():
    nc = bacc.Bacc("TRN2", target_bir_lowering=False, debug=False)

    x_d = nc.dram_tensor("x", [N, DIN], F32, kind="ExternalInput").ap()
    adj_d = nc.dram_tensor("adj", [N, N], mybir.dt.int32, kind="ExternalInput").ap()
    Wp_d = nc.dram_tensor("Wp", [DIN, F], F32, kind="ExternalInput").ap()
    bp_d = nc.dram_tensor("bp", [F], F32, kind="ExternalInput").ap()
    Wh_d = nc.dram_tensor("W_heads", [L, H, F, F], F32, kind="ExternalInput").ap()
    ah_d = nc.dram_tensor("a_heads", [L, H, 2 * F], F32, kind="ExternalInput").ap()
    Wo_d = nc.dram_tensor("W_out", [L, H * F, F], F32, kind="ExternalInput").ap()
    ao_d = nc.dram_tensor("a_out", [L, 2 * F], F32, kind="ExternalInput").ap()
    g_d = nc.dram_tensor("ln_g", [L, F], F32, kind="ExternalInput").ap()
    b_d = nc.dram_tensor("ln_b", [L, F], F32, kind="ExternalInput").ap()
    out_d = nc.dram_tensor("out", [N, F], F32, kind="ExternalOutput").ap()
    # DRAM bounce buffer for E1b row broadcasts (one per layer, 2 slots)
    ebl_d = [nc.dram_tensor(f"eblk{l}", [16, N], BF16, kind="ExternalOutput").ap()
             for l in range(L)]

    with tile.TileContext(nc) as tc, ExitStack() as ctx:
        const = ctx.enter_context(tc.tile_pool(name="const", bufs=1))
        sx = ctx.enter_context(tc.tile_pool(name="sx", bufs=2))
        sproj = ctx.enter_context(tc.tile_pool(name="sproj", bufs=10))
        sbcast = ctx.enter_context(tc.tile_pool(name="sbcast", bufs=10))
        sexp = ctx.enter_context(tc.tile_pool(name="sexp", bufs=4))
        smulti = ctx.enter_context(tc.tile_pool(name="smulti", bufs=9))
        sbig = ctx.enter_context(tc.tile_pool(name="sbig", bufs=3))
        srow = ctx.enter_context(tc.tile_pool(name="srow", bufs=5))
        shd = ctx.enter_context(tc.tile_pool(name="shd", bufs=4))
        smask = ctx.enter_context(tc.tile_pool(name="smask", bufs=4))
        pou = ctx.enter_context(tc.tile_pool(name="pou", bufs=3, space="PSUM"))
        pmisc = ctx.enter_context(tc.tile_pool(name="pmisc", bufs=2, space="PSUM"))
        prow = ctx.enter_context(tc.tile_pool(name="prow", bufs=3, space="PSUM"))

        # ---------------- constants ----------------
        ones_row = const.tile([1, N], F32)
        nc.vector.memset(ones_row, 1.0)
        ones_row_bf = const.tile([1, N], BF16)
        nc.vector.memset(ones_row_bf, 1.0)
        ones_col = const.tile([128, 1], F32)
        nc.vector.memset(ones_col, 1.0)
        ones_col_bf = const.tile([128, 1], BF16)
        nc.vector.memset(ones_col_bf, 1.0)
        ident = const.tile([128, 128], F32)
        make_identity(nc, ident)
        ident_bf = const.tile([128, 128], BF16)
        nc.vector.tensor_copy(ident_bf, ident)
        eps1 = const.tile([1, 1], F32)
        nc.vector.memset(eps1, LN_EPS)

        Wp_sb = const.tile([DIN, F], F32)
        nc.sync.dma_start(Wp_sb, Wp_d)
        bp_col = const.tile([F, 1], F32)
        nc.sync.dma_start(bp_col, bp_d.rearrange("(f one) -> f one", one=1))
        x_chunks = []
        for c in range(NCHUNK):
            xc = shd.tile([128, DIN], F32, tag="xchunk")
            nc.sync.dma_start(xc, x_d[bass.ts(c, 128), :])
            x_chunks.append(xc)

        # per-layer weight loads: gpsimd swdge DMAs cast f32->bf16 directly
        Wh_ball = [const.tile([F, H, F], BF16, name=f"WhB{l}") for l in range(L)]
        for l in range(L):
            nc.gpsimd.dma_start(Wh_ball[l], Wh_d[l].rearrange("h i o -> i h o"))
        Wh_bf = [[Wh_ball[l][:, h, :] for h in range(H)] for l in range(L)]

        ah_ball = const.tile([F, L * H, 2], BF16)
        nc.gpsimd.dma_start(ah_ball, ah_d.rearrange("l h (t f) -> f (l h) t", t=2))
        ah_bf = [[ah_ball[:, l * H + h, :] for h in range(H)] for l in range(L)]

        Wo_ball = [const.tile([128, H, F], BF16, name=f"WoB{l}") for l in range(L)]
        for l in range(L):
            nc.gpsimd.dma_start(Wo_ball[l], Wo_d[l].rearrange("(c p) f -> p c f", p=128))
        Wo_bf = Wo_ball

        ao_ball = const.tile([F, L, 2], BF16)
        nc.gpsimd.dma_start(ao_ball, ao_d.rearrange("l (t f) -> f l t", t=2))
        ao_bf = [ao_ball[:, l, :] for l in range(L)]

        g_all = const.tile([1, L, F], F32)
        nc.scalar.dma_start(g_all, g_d.rearrange("l f -> (l f)").rearrange(
            "(one l f) -> one l f", one=1, l=L))
        b_all = const.tile([1, L, F], F32)
        nc.scalar.dma_start(b_all, b_d.rearrange("l f -> (l f)").rearrange(
            "(one l f) -> one l f", one=1, l=L))
        gc_all = const.tile([F, L], F32)
        nc.scalar.dma_start(gc_all, g_d.rearrange("l f -> f l"))
        g_row = [g_all[:, l, :] for l in range(L)]
        b_row = [b_all[:, l, :] for l in range(L)]
        g_col = [gc_all[:, l:l + 1] for l in range(L)]
        gb_bf = const.tile([1, 2 * L, F], BF16)
        nc.vector.tensor_copy(gb_bf[:, 0:L, :], g_all)
        nc.vector.tensor_copy(gb_bf[:, L:2 * L, :], b_all)
        g_row_bf = [gb_bf[:, l, :] for l in range(L)]
        b_row_bf = [gb_bf[:, L + l, :] for l in range(L)]

        # ------------- WhT (transposed head weights) + Wtilde = W @ a -------
        WhT_ball = [const.tile([F, H, F], BF16, name=f"WhT{l}") for l in range(L)]
        for l in range(L):
            for h in range(H):
                pt = pou.tile([128, 128], BF16, tag="oU")
                nc.tensor.transpose(pt, Wh_bf[l][h], ident_bf)
                if h % 2 == 0:
                    nc.scalar.activation(WhT_ball[l][:, h, :], pt, AF.Copy)
                else:
                    nc.vector.tensor_copy(WhT_ball[l][:, h, :], pt)
        Wt_bf = [const.tile([F, 2 * H], BF16, name=f"Wt{l}") for l in range(L)]
        for l in range(L):
            pw = prow.tile([128, 2 * H], F32, tag="prow")
            for h in range(H):
                nc.tensor.matmul(pw[:, 2 * h:2 * h + 2], WhT_ball[l][:, h, :],
                                 ah_bf[l][h], start=True, stop=True)
            nc.scalar.activation(Wt_bf[l], pw, AF.Copy)

        # ---------------- x -> xT, input projection ----------------
        xT = const.tile([DIN, N], F32)
        ph = pmisc.tile([128, N], F32, tag="pbig")
        hT = sbig.tile([128, N], F32, tag="hT")
        hT_bf = sbig.tile([128, N], BF16, tag="hTb", bufs=2)
        for c in range(NCHUNK):
            pt = pmisc.tile([DIN, 128], F32, tag="pbig")
            nc.tensor.transpose(pt, x_chunks[c], ident)
            nc.scalar.activation(xT[:, bass.ts(c, 128)], pt, AF.Copy)
            nc.tensor.matmul(ph[:, bass.ts(c, 128)], Wp_sb, xT[:, bass.ts(c, 128)],
                             start=True, stop=True)
            nc.scalar.activation(hT[:, bass.ts(c, 128)], ph[:, bass.ts(c, 128)],
                                 AF.Relu, bias=bp_col)
            nc.vector.tensor_copy(hT_bf[:, bass.ts(c, 128)], hT[:, bass.ts(c, 128)])

        # ---------------- adj -> maskT (bf16, transposed) ----------------
        adj_f = []
        for r in range(NCHUNK):
            ai = shd.tile([128, N], mybir.dt.int32, tag="adji")
            eng = nc.scalar if r % 2 == 0 else nc.sync
            eng.dma_start(ai, adj_d[bass.ts(r, 128), :])
            af = smask.tile([128, N], BF16, tag="adjf")
            nc.vector.tensor_copy(af, ai)
            adj_f.append(af)
        maskT = [const.tile([128, N], BF16, name=f"maskT{c}") for c in range(NCHUNK)]
        for r in range(NCHUNK):
            for c in range(NCHUNK):
                pm = pmisc.tile([128, 128], BF16, tag="pbig")
                nc.tensor.transpose(pm, adj_f[r][:, bass.ts(c, 128)], ident_bf)
                nc.scalar.activation(maskT[c][:, bass.ts(r, 128)], pm, AF.Copy)

        # ------------- attention body (shared by heads & out-att) -----------
        def attention(e1b_sb, e2i_cols, e2_cols, projNp, gp_stt, out_f32=False):
            """e1b_sb: [128,N] bf16 bcast of E1b row.  e2i_cols/e2_cols: 4
            [128,1] col APs (E2inv f32 / E2 bf16).  projNp: [128,NCHUNK,128]
            bf16 AV lhsT already scaled by E2[m].  Returns outT = pou/den."""
            t_m = sexp.tile([128, NCHUNK, N], BF16, tag="t_m", bufs=2)
            s_t = sexp.tile([128, NCHUNK, N], BF16, tag="s_t")
            for c in range(NCHUNK):
                nc.vector.tensor_scalar_max(t_m[:, c, :], e1b_sb, e2i_cols[c])
                nc.vector.tensor_tensor(s_t[:, c, :], t_m[:, c, :], maskT[c],
                                        OP.mult)
            den_ps = prow.tile([1, N], F32, tag="prow")
            for c in range(NCHUNK):
                nc.tensor.matmul(den_ps, e2_cols[c], s_t[:, c, :],
                                 start=(c == 0), stop=(c == NCHUNK - 1))
            pou_ps = pou.tile([128, N], F32, tag="oU")
            for c in range(NCHUNK):
                nc.tensor.matmul(pou_ps, projNp[:, c, :], s_t[:, c, :],
                                 start=(c == 0), stop=(c == NCHUNK - 1))
            rrow = srow.tile([1, N], F32, tag="rrowf")
            nc.vector.reciprocal_approx_fast(rrow, den_ps)
            rrow_bf = srow.tile([1, N], BF16, tag="rrowb")
            nc.vector.tensor_copy(rrow_bf, rrow)
            rep_ps = prow.tile([128, N], F32, tag="prow")
            nc.tensor.matmul(rep_ps, ones_row_bf[:, 0:128], rrow_bf,
                             start=True, stop=True)
            rep = sbcast.tile([128, N], BF16, tag="rep", bufs=6)
            nc.scalar.activation(rep, rep_ps, AF.Copy)
            pou_bf = shd.tile([128, N], BF16, tag="poubf")
            nc.scalar.activation(pou_bf, pou_ps, AF.Copy)
            outT = sbig.tile([128, N], F32 if out_f32 else BF16, tag="outT",
                             bufs=4)
            nc.vector.tensor_tensor(outT, pou_bf, rep, OP.mult)
            return outT

        # ---------------- layers ----------------
        for l in range(L):
            residT = hT
            # --- rows for all heads: s12[2h] = s1_h, s12[2h+1] = s2_h
            s12_ps = prow.tile([2 * H, N], F32, tag="prow")
            nc.tensor.matmul(s12_ps, Wt_bf[l], hT_bf, start=True, stop=True)
            Eblk = sx.tile([16, N], BF16, tag="Eblk")   # exp(+beta*s): rows 2h = E1b
            nc.scalar.activation(Eblk, s12_ps, AF.Exp, scale=BETA)
            Xneg = sx.tile([16, N], BF16, tag="Xneg")   # exp(-beta*s): 2h+1 = E2inv
            nc.scalar.activation(Xneg, s12_ps, AF.Exp, scale=-BETA)
            Xpos = sx.tile([16, N], BF16, tag="Xpos")   # exp(s): 2h+1 = E2
            nc.scalar.activation(Xpos, s12_ps, AF.Exp, scale=1.0)
            # E1b broadcasts: one DRAM bounce write of all rows, then one
            # stride-0 broadcast read per head, spread across DMA queues
            dmaq = [nc.sync, nc.scalar, nc.gpsimd]
            nc.sync.dma_start(ebl_d[l], Eblk)
            e1b = []
            for h in range(H):
                row = ebl_d[l][2 * h, :]
                src_bc = bass.AP(tensor=row.tensor, offset=row.offset,
                                 ap=[[0, 128], [1, N]])
                eb = sbcast.tile([128, N], BF16, tag="e1b")
                dmaq[h % 3].dma_start(eb, src_bc)
                e1b.append(eb)
            # --- columns: transpose Xneg/Xpos -> Xcols [128, 8*16]
            xc_ps = prow.tile([128, 8 * 16], BF16, tag="prow")
            for c in range(NCHUNK):
                nc.tensor.transpose(xc_ps[:, c * 16:(c + 1) * 16],
                                    Xneg[:, bass.ts(c, 128)], ident_bf[0:16, 0:16])
                nc.tensor.transpose(xc_ps[:, 64 + c * 16:64 + (c + 1) * 16],
                                    Xpos[:, bass.ts(c, 128)], ident_bf[0:16, 0:16])
            Xcols = sx.tile([128, 8 * 16], F32, tag="Xcols")
            nc.scalar.activation(Xcols, xc_ps, AF.Copy)
            Xcols_bf = sx.tile([128, 8 * 16], BF16, tag="Xcolsb")
            nc.vector.tensor_copy(Xcols_bf, xc_ps)

            def e2i_col(h, c):
                j = c * 16 + 2 * h + 1
                return Xcols[:, j:j + 1]

            def e2_col(h, c):
                j = 64 + c * 16 + 2 * h + 1
                return Xcols[:, j:j + 1]

            def e2_col_bf(h, c):
                j = 64 + c * 16 + 2 * h + 1
                return Xcols_bf[:, j:j + 1]

            # --- projN per head (scaled by E2[m])
            projNp = []
            for h in range(H):
                pN = pmisc.tile([128, N], F32, tag="pbig")
                for c in range(NCHUNK):
                    nc.tensor.matmul(pN[:, bass.ts(c, 128)], hT_bf[:, bass.ts(c, 128)],
                                     Wh_bf[l][h], start=True, stop=True)
                pp = sproj.tile([128, NCHUNK, 128], BF16, tag="projNp")
                for c in range(NCHUNK):
                    nc.scalar.activation(pp[:, c, :], pN[:, bass.ts(c, 128)],
                                         AF.Identity, scale=e2_col(h, c))
                projNp.append(pp)
            # --- attention per head + ELU
            multiT = []
            for h in range(H):
                outT = attention(
                    e1b[h],
                    [e2i_col(h, c) for c in range(NCHUNK)],
                    [e2_col_bf(h, c) for c in range(NCHUNK)],
                    projNp[h], GP_STT)
                ex = shd.tile([128, N], BF16, tag="elu_ex")
                nc.scalar.activation(ex, outT, AF.Exp)
                eng = nc.gpsimd if GP_ELU else nc.vector
                eng.tensor_scalar(ex, ex, 1.0, -1.0, OP.min, OP.add)
                mh = smulti.tile([128, N], BF16, tag="multi")
                eng2 = nc.gpsimd if GP_EMAX else nc.vector
                eng2.tensor_tensor(mh, outT, ex, OP.max)
                multiT.append(mh)

            # --- W_out projection
            ph2 = pou.tile([128, N], F32, tag="oU")
            for h in range(H):
                nc.tensor.matmul(ph2, Wo_bf[l][:, h, :], multiT[h],
                                 start=(h == 0), stop=(h == H - 1))
            h2_bf = sbig.tile([128, N], BF16, tag="h2b", bufs=2)
            nc.scalar.activation(h2_bf, ph2, AF.Copy)

            # --- single out-attention
            s12o_ps = prow.tile([2, N], F32, tag="prow")
            nc.tensor.matmul(s12o_ps, ao_bf[l], h2_bf, start=True, stop=True)
            Xo_b = sx.tile([2, N], BF16, tag="Xo_b")    # row 0 = E1b_o
            nc.scalar.activation(Xo_b, s12o_ps, AF.Exp, scale=BETA)
            Xo_nb = sx.tile([2, N], BF16, tag="Xo_nb")  # row 1 = E2inv_o
            nc.scalar.activation(Xo_nb, s12o_ps, AF.Exp, scale=-BETA)
            Xo_1 = sx.tile([2, N], BF16, tag="Xo_1")    # row 1 = E2_o
            nc.scalar.activation(Xo_1, s12o_ps, AF.Exp, scale=1.0)
            xo_ps = prow.tile([128, 16], BF16, tag="prow")
            for c in range(NCHUNK):
                nc.tensor.transpose(xo_ps[:, c * 2:(c + 1) * 2],
                                    Xo_nb[:, bass.ts(c, 128)], ident_bf[0:2, 0:2])
                nc.tensor.transpose(xo_ps[:, 8 + c * 2:8 + (c + 1) * 2],
                                    Xo_1[:, bass.ts(c, 128)], ident_bf[0:2, 0:2])
            Xoc = sx.tile([128, 16], F32, tag="Xoc")
            nc.scalar.activation(Xoc, xo_ps, AF.Copy)
            Xoc_bf = sx.tile([128, 16], BF16, tag="Xocb")
            nc.vector.tensor_copy(Xoc_bf, xo_ps)
            # E1b_o broadcast via PE rank-1 (low latency; PE idle here)
            ebo_ps = pmisc.tile([128, N], F32, tag="pbig")
            nc.tensor.matmul(ebo_ps, ones_row_bf[:, 0:128], Xo_b[0:1, :], start=True, stop=True)
            e1bo = sbcast.tile([128, N], BF16, tag="e1b")
            nc.scalar.activation(e1bo, ebo_ps, AF.Copy)
            # h2N via transposes, scaled by E2o[m]
            h2n_ps = pmisc.tile([128, N], BF16, tag="pbig")
            for c in range(NCHUNK):
                nc.tensor.transpose(h2n_ps[:, bass.ts(c, 128)],
                                    h2_bf[:, bass.ts(c, 128)], ident_bf)
            h2Np = sproj.tile([128, NCHUNK, 128], BF16, tag="projNp")
            for c in range(NCHUNK):
                nc.scalar.activation(h2Np[:, c, :], h2n_ps[:, bass.ts(c, 128)],
                                     AF.Identity, scale=Xoc[:, 8 + c * 2 + 1:8 + c * 2 + 2])
            outsT = attention(
                e1bo,
                [Xoc[:, c * 2 + 1:c * 2 + 2] for c in range(NCHUNK)],
                [Xoc_bf[:, 8 + c * 2 + 1:8 + c * 2 + 2] for c in range(NCHUNK)],
                h2Np, 0, out_f32=True)

            # ---- residual + LN over partition dim ----
            xs = sbig.tile([128, N], F32, tag="xs", bufs=2)
            nc.vector.tensor_tensor(xs, outsT, residT, OP.add)
            xs_bf = sbig.tile([128, N], BF16, tag="xsqb", bufs=2)
            nc.vector.tensor_copy(xs_bf, xs)
            xsq = sbig.tile([128, N], BF16, tag="xsq", bufs=2)
            nc.scalar.activation(xsq, xs, AF.Square)
            pmu = prow.tile([1, N], F32, tag="prow")
            nc.tensor.matmul(pmu, ones_col_bf, xs_bf, start=True, stop=True)
            psq = prow.tile([1, N], F32, tag="prow")
            nc.tensor.matmul(psq, ones_col_bf, xsq, start=True, stop=True)
            mu = srow.tile([1, N], F32, tag="rowL")
            nc.vector.tensor_scalar_mul(mu, pmu, 1.0 / F)
            msq = srow.tile([1, N], F32, tag="rowL")
            nc.vector.tensor_scalar_mul(msq, psq, 1.0 / F)
            mu2 = srow.tile([1, N], F32, tag="rowL")
            nc.vector.tensor_tensor(mu2, mu, mu, OP.mult)
            var = srow.tile([1, N], F32, tag="rowL")
            nc.vector.tensor_tensor(var, msq, mu2, OP.subtract)
            lnv = srow.tile([1, N], F32, tag="rowL")
            nc.scalar.activation(lnv, var, AF.Ln, bias=eps1)
            rstd = srow.tile([1, N], F32, tag="rowL")
            nc.scalar.activation(rstd, lnv, AF.Exp, scale=-0.5)
            mr = srow.tile([1, N], F32, tag="rowL")
            nc.vector.tensor_tensor(mr, mu, rstd, OP.mult)
            r2 = srow.tile([1, N], BF16, tag="rowLb")
            nc.vector.tensor_scalar_mul(r2, mr, -1.0)
            rstd_bf = srow.tile([1, N], BF16, tag="rowLb")
            nc.vector.tensor_copy(rstd_bf, rstd)
            paff = pmisc.tile([128, N], F32, tag="pbig")
            nc.tensor.matmul(paff, g_row_bf[l], r2, start=True, stop=False)
            nc.tensor.matmul(paff, b_row_bf[l], ones_row_bf, start=False, stop=True)
            prs = pmisc.tile([128, N], F32, tag="pbig")
            nc.tensor.matmul(prs, g_row_bf[l], rstd_bf,
                             start=True, stop=True)
            rep_grstd = sbig.tile([128, N], F32, tag="repo", bufs=2)
            nc.scalar.activation(rep_grstd, prs, AF.Copy)
            y = sbig.tile([128, N], F32, tag="y", bufs=2)
            nc.vector.tensor_tensor(y, xs, rep_grstd, OP.mult)
            hT_new = sbig.tile([128, N], F32, tag="hT")
            nc.vector.tensor_tensor(hT_new, y, paff, OP.add)
            if l < L - 1:
                nc.vector.tensor_scalar_max(hT_new, hT_new, 0.0)
            hT = hT_new
            if l < L - 1:
                hT_bf = sbig.tile([128, N], BF16, tag="hTb", bufs=2)
                nc.vector.tensor_copy(hT_bf, hT)

        # ---------------- output: transpose back ----------------
        for c in range(NCHUNK):
            po = pmisc.tile([128, 128], F32, tag="pbig")
            nc.tensor.transpose(po, hT[:, bass.ts(c, 128)], ident)
            osb = shd.tile([128, 128], F32, tag="osb")
            nc.scalar.activation(osb, po, AF.Copy)
            nc.sync.dma_start(out_d[bass.ts(c, 128), :], osb)

    nc.compile()
    return nc


def _get_nc():
    if "nc" not in _CACHE:
        _CACHE["nc"] = build_nc()
    return _CACHE["nc"]


def kernel(**inputs) -> np.ndarray:
    nc = _get_nc()
    shared = {k: np.ascontiguousarray(np.asarray(inputs[k], dtype=np.float32))
              for k in ("Wp", "bp", "W_heads", "a_heads", "W_out", "a_out",
                        "ln_g", "ln_b")}
    x = np.asarray(inputs["x"], dtype=np.float32)
    adj = np.asarray(inputs["adj"], dtype=np.int32)
    in_maps = [dict(x=np.ascontiguousarray(x[b]),
                    adj=np.ascontiguousarray(adj[b]), **shared)
               for b in range(B)]
    res = run_bass_kernel_spmd(nc, in_maps, core_ids=list(range(B)))
    return np.stack([res.results[b]["out"] for b in range(B)])


if __name__ == "__main__":
    rng = np.random.default_rng(0)
    inputs = dict(
        x=rng.normal(size=(B, N, DIN)).astype(np.float32),
        adj=rng.integers(0, 2, size=(B, N, N)).astype(np.int32),
        Wp=(rng.normal(size=(DIN, F)) * 0.12).astype(np.float32),
        bp=np.zeros(F, dtype=np.float32),
        W_heads=(rng.normal(size=(L, H, F, F)) * 0.08).astype(np.float32),
        a_heads=(rng.normal(size=(L, H, 2 * F)) * 0.08).astype(np.float32),
        W_out=(rng.normal(size=(L, H * F, F)) * 0.03).astype(np.float32),
        a_out=(rng.normal(size=(L, 2 * F)) * 0.08).astype(np.float32),
        ln_g=np.ones((L, F), dtype=np.float32),
        ln_b=np.zeros((L, F), dtype=np.float32),
    )
    out = kernel(**inputs)
    print("out", out.shape, out.dtype, np.abs(out).max())
